# revision 62
# baseline (speedup 1.0000x reference)
"""Trainium2 Bass kernel for nn_BasicCGInducer (CKY inside algorithm for a
categorial-grammar inducer).

Strategy (8 NeuronCores):
  - Data-parallel over sentences: core j handles sentences 4j..4j+3.
  - Emission log-partition ([C,V] softmax denominator) is tensor-parallel
    over vocab: each core exps a 4000-column V-shard, one AllReduce of [C].
  - CKY inside pass runs per-core in scaled-exp space with POWER-OF-TWO
    integer span scales: the per-level rescales are pure DVE bit/int ops
    (no activation-table Exp/Ln in the loop), and all chart data, rule
    tables and products are fp16 to engage the DVE 2x/4x perf modes.
  - All matmuls (emission, split-MLP, beta1) run in bf16.

kernel(**inputs) takes FULL inputs, shards on host, runs one SPMD bass
program on cores 0-7, and reassembles the [32] output.
"""
import sys
import contextlib

sys.path.insert(0, "/opt/trn_rl_repo")

import numpy as np

import concourse.bass as bass
import concourse.bacc as bacc
import concourse.mybir as mybir
import concourse.tile as tile
from concourse.ap import AP
from concourse import bass_utils

F32 = mybir.dt.float32
F16 = mybir.dt.float16
BF16 = mybir.dt.bfloat16
I32 = mybir.dt.int32
ALU = mybir.AluOpType
ACTF = mybir.ActivationFunctionType
AXIS = mybir.AxisListType

# ---------------------------------------------------------------- constants
P4 = 4          # primitive cats
NF = 36         # non-functor cats
C = 2596        # total cats
CP = 2688       # padded C (21 * 128)
NT = CP // 128  # 21 c-tiles
D = 64
B = 32          # total sentences
NCORES = 8
BLOC = B // NCORES  # 4 sentences per core
V = 32000
BLK = 72        # chart block: [0:36 chart | 36:40 pad | 40:56 FB | 56:72 FA]
NEGB = -30.0    # bias for padded vocab columns (exp(-30) ~ 0 vs Z ~ 6e3)
GBOOST = 10     # G tables carry 2^GBOOST; span scale compensates
LN2 = 0.6931471805599453
LOG2E = 1.4426950408889634
# Schraudolph fast-exp: bitcast_f32(int32(x * 2^23/ln2 + b)), mean-unbiased b
SCH_A = 12102203.161561485
SCH_B = 1064866805.0
ADJK = 16       # 2^ADJK boost on the emission adj factor (lse ~ 10.9 nats)


class Cfg:
    def __init__(self, n=32, v_loc=4000, n_cores=8):
        self.n = n                      # sentence length
        self.v_loc = v_loc              # vocab shard per core
        self.v_pad = ((v_loc + 511) // 512) * 512
        self.n_cores = n_cores
        self.pairs = 4 * n              # (i, b) pairs on partitions


# ------------------------------------------------------------ functor maps
def lf_block_offsets(op):
    """c = off + {A: 4r+a | B: 32r+(a-4) | C: 36(r-4)+a} per derivation of
    the deterministic functor-id tables. op=0 -> l_functors, 1 -> r_functors."""
    return {
        "A": 4 + 16 * op,            # res<4, arg<4 : c = A + 4*res + arg
        "B": 36 + 1280 * op,         # res<4, arg>=4: c = B + 32*res + (arg-4)
        "C": 164 + 1280 * op,        # res>=4      : c = C0 + 36*(res-4) + arg
    }


def check_functor_tables(l_functors, r_functors):
    for op, tab in ((0, l_functors), (1, r_functors)):
        off = lf_block_offsets(op)
        exp = np.zeros((NF, NF), np.int64)  # [arg, res]
        for res in range(NF):
            for arg in range(NF):
                if res < P4 and arg < P4:
                    exp[arg, res] = off["A"] + 4 * res + arg
                elif res < P4:
                    exp[arg, res] = off["B"] + 32 * res + (arg - 4)
                else:
                    exp[arg, res] = off["C"] + 36 * (res - 4) + arg
        assert np.array_equal(np.asarray(tab, np.int64), exp), (
            f"functor table structure mismatch (op={op})")


# ---------------------------------------------------------------- AP helper
def mk(t, parts, off, dims, base_part=0):
    """Raw AP on tile t: partition range [base_part, base_part+parts),
    free offset `off` (elements), extra free dims [[step, count], ...]."""
    w = t.ap[0][0]
    return AP(t.tensor, t.offset + base_part * w + off, [[w, parts]] + dims)


# ============================================================ device program
def build_program(cfg: Cfg):
    nc = bacc.Bacc("TRN2", target_bir_lowering=False, debug=False,
                   num_devices=cfg.n_cores)
    d = {
        "ntembT": nc.dram_tensor("ntembT", [65, CP], BF16,
                                 kind="ExternalInput"),
        "vocabW": nc.dram_tensor("vocabW", [65, cfg.v_pad], BF16,
                                 kind="ExternalInput"),
        "wordW": nc.dram_tensor("wordW", [66, cfg.pairs], BF16,
                                kind="ExternalInput"),
        "mlpW": nc.dram_tensor("mlpW", [64, 322], BF16, kind="ExternalInput"),
        "mlpB": nc.dram_tensor("mlpB", [64, 8], F32, kind="ExternalInput"),
        "ruleWb": nc.dram_tensor("ruleWb", [36, 144], F32,
                                 kind="ExternalInput"),
        "smallv": nc.dram_tensor("smallv", [1, 16], F32,
                                 kind="ExternalInput"),
        "out": nc.dram_tensor("out_nll", [BLOC, 1], F32,
                              kind="ExternalOutput"),
    }
    with tile.TileContext(nc) as tc:
        with nc.allow_low_precision(reason="fp16 CKY inside pass"):
            _trace(tc, cfg, d)
    nc.compile()
    return nc


def _trace(tc, cfg, d):
    nc = tc.nc
    n, PAIRS, VP = cfg.n, cfg.pairs, cfg.v_pad
    NV = VP // 512                    # 512-col v-tiles per core
    NHALF = (NV + 3) // 4             # ACT chunks of up to 4 v-tiles
    HW = CP // 2                      # MLP half width (1344)

    es = contextlib.ExitStack()
    keep = es.enter_context(tc.tile_pool(name="keep", bufs=1))
    dram = es.enter_context(tc.tile_pool(name="dram", bufs=1, space="DRAM"))

    # ---------------- long-lived tensors
    chartA = keep.tile([PAIRS, (n + 1) * BLK], F16)
    chartE = keep.tile([PAIRS, (n + 1) * BLK], F16)
    scaleA = keep.tile([PAIRS, n + 2], F32)   # col L = scale of chart[L]
    scaleEr = keep.tile([PAIRS, n + 2], F32)  # col n-m = scale of end block m
    WA = keep.tile([PAIRS, 1296], F16)
    WB = keep.tile([PAIRS, 1296], F16)
    glR = keep.tile([128, 1296], F16)
    grR = keep.tile([128, 1296], F16)
    gfb = keep.tile([128, 32], F16)           # [0:16 grR 4x4 | 16:32 glR 4x4]
    M1f = keep.tile([PAIRS, 4], F32)          # 0: M1i (int-val), 1: -ln2*M1i
    m1i = keep.tile([PAIRS, 2], I32)
    mlpB = keep.tile([64, 8], F32)
    smallv = keep.tile([1, 16], F32)
    sumexp_parts = keep.tile([128, NT * NHALF], F32)
    sumexp_loc = keep.tile([128, NT], F32)
    sumexp_g = keep.tile([128, NT], F32)
    lse21 = keep.tile([128, NT], F32)
    s0E = keep.tile([1, NF], F32)
    db = keep.tile([1, 2], F32)
    rsRep = keep.tile([4, 4], F16)
    fin = keep.tile([4, 8], F32)
    finh = keep.tile([4, 4], F16)
    expMask = keep.tile([128, 1], I32)
    nc.gpsimd.memset(expMask[:], 0x7F800000)

    nc.sync.dma_start(mlpB[:], d["mlpB"][:])
    nc.sync.dma_start(smallv[:], d["smallv"][:])
    nc.gpsimd.memset(chartA[:], 0.0)
    nc.gpsimd.memset(chartE[:], 0.0)
    nc.gpsimd.memset(scaleA[:], 0.0)
    nc.gpsimd.memset(scaleEr[:], 0.0)

    ph1 = contextlib.ExitStack()
    p1 = ph1.enter_context(tc.tile_pool(name="ph1", bufs=1))
    ntembT = p1.tile([65, CP], BF16)
    vocabW = p1.tile([65, VP], BF16)
    wordW = p1.tile([66, PAIRS], BF16)
    mlpW = p1.tile([64, 322], BF16)
    ruleWb = p1.tile([36, 144], F32)
    lse_row = p1.tile([1, CP], F32)
    adj = p1.tile([1, CP], F32)
    spN = p1.tile([1, CP], F32)
    beta1E = p1.tile([PAIRS, CP], F16)
    ruleflat = p1.tile([1, 36 * 72], F32)

    nc.sync.dma_start(ntembT[:], d["ntembT"][:])
    nc.sync.dma_start(vocabW[:], d["vocabW"][:])
    nc.sync.dma_start(wordW[:], d["wordW"][:])
    nc.sync.dma_start(mlpW[:], d["mlpW"][:])
    nc.sync.dma_start(ruleWb[:], d["ruleWb"][:])

    # =======================================================================
    # Phase 1: emission partition function. exp chunks split between the
    # scalar engine (table exp + accum) and DVE (Schraudolph bitcast exp).
    # =======================================================================
    with tc.tile_pool(name="psum_e", bufs=2, space="PSUM") as pse, \
         tc.tile_pool(name="scr_e", bufs=2) as scre:
        for ct in range(NT):
            for h in range(NHALF):
                vt0 = h * 4
                nvt = min(4, NV - vt0)
                pt = pse.tile([128, 512 * nvt], F32, tag="pse")
                for vt in range(nvt):
                    # 64-row emb matmul + 1-row bias matmul: a 65-row
                    # contraction pays the 128-row weight-tile rate (2x)
                    nc.tensor.matmul(
                        pt[:, vt * 512:(vt + 1) * 512],
                        ntembT[0:64, ct * 128:(ct + 1) * 128],
                        vocabW[0:64, (vt0 + vt) * 512:(vt0 + vt + 1) * 512],
                        start=True, stop=False)
                    nc.tensor.matmul(
                        pt[:, vt * 512:(vt + 1) * 512],
                        ntembT[64:65, ct * 128:(ct + 1) * 128],
                        vocabW[64:65, (vt0 + vt) * 512:(vt0 + vt + 1) * 512],
                        start=False, stop=True)
                acc = sumexp_parts[:, ct * NHALF + h:ct * NHALF + h + 1]
                ci = ct * NHALF + h
                if (ci * 15) % 42 >= 15:  # 27: scalar-engine exp + accum
                    sce = scre.tile([128, 512 * 4], F32, tag="scre")
                    nc.scalar.activation(
                        sce[:, 0:512 * nvt], pt[:], ACTF.Exp, accum_out=acc)
                else:                 # 14: DVE Schraudolph exp + sum
                    sci = scre.tile([128, 512 * 4], I32, tag="scri")
                    nc.vector.tensor_scalar(
                        sci[:, 0:512 * nvt], pt[:], SCH_A, SCH_B,
                        op0=ALU.mult, op1=ALU.add)
                    nc.vector.tensor_reduce(
                        acc, sci[:, 0:512 * nvt].bitcast(F32),
                        axis=AXIS.X, op=ALU.add)

    if NHALF > 1:
        nc.vector.tensor_reduce(
            sumexp_loc[:],
            mk(sumexp_parts, 128, 0, [[NHALF, NT], [1, NHALF]]),
            axis=AXIS.X, op=ALU.add)
    else:
        nc.vector.tensor_copy(sumexp_loc[:], sumexp_parts[:, 0:NT])

    # AllReduce over cores via DRAM bounce
    cc_in = dram.tile([128, NT], F32)
    cc_out = dram.tile([128, NT], F32)
    nc.sync.dma_start(cc_in[:], sumexp_loc[:])
    nc.gpsimd.collective_compute(
        "AllReduce", ALU.add,
        replica_groups=[list(range(cfg.n_cores))],
        ins=[cc_in[:].opt()], outs=[cc_out[:].opt()])
    # the collective-dependent lse/adj tail is issued after phase 3's
    # collective-independent work so no engine queue blocks on it.

    # =======================================================================
    # Phase 2: split MLP (transposed layout hT [64, *]), rule tables, root
    # =======================================================================
    nc.vector.tensor_tensor(db[:, 0:1], smallv[:, 0:1], smallv[:, 1:2],
                            op=ALU.subtract)
    nc.vector.tensor_scalar_mul(db[:, 1:2], db[:, 0:1], -1.0)

    with tc.tile_pool(name="mlp", bufs=1) as mlp:
        hA = mlp.tile([64, HW], BF16, tag="hA")
        hB = mlp.tile([64, HW], BF16, tag="hB")
        hC = mlp.tile([64, HW], BF16, tag="hC")
        s_rows = mlp.tile([2, HW], F32, tag="srows")
        w1 = mlp.tile([1, HW], F32, tag="w1")
        w2 = mlp.tile([1, HW], F32, tag="w2")
        w3 = mlp.tile([1, HW], F32, tag="w3")

        for half in range(2):
            base = half * HW

            def dense_relu(dst, col0, rhs, bias_col, res_add=None, rb=0,
                           relu=True):
                with tc.tile_pool(name="psum_m", bufs=2,
                                  space="PSUM") as psm:
                    for c0 in range(0, HW, 512):
                        c1 = min(c0 + 512, HW)
                        pm = psm.tile([64, 512], F32, tag="psm")
                        nc.tensor.matmul(pm[:, 0:c1 - c0],
                                         mlpW[:, col0:col0 + 64],
                                         rhs[0:64, rb + c0:rb + c1],
                                         start=True, stop=True)
                        nc.scalar.activation(
                            dst[:, c0:c1], pm[:, 0:c1 - c0],
                            ACTF.Relu if relu else ACTF.Identity,
                            bias=mlpB[0:64, bias_col:bias_col + 1])
                        if res_add is not None:
                            nc.vector.tensor_tensor(
                                dst[:, c0:c1], dst[:, c0:c1],
                                res_add[:, c0:c1], op=ALU.add)

            dense_relu(hA, 0, ntembT, 0, rb=base,
                       relu=False)                      # h1 (linear)
            dense_relu(hB, 64, hA, 1)                   # t = relu(h1 W + b)
            dense_relu(hC, 128, hB, 2, res_add=hA)      # h2
            dense_relu(hB, 192, hC, 3)                  # t2
            dense_relu(hA, 256, hB, 4, res_add=hC)      # h3

            with tc.tile_pool(name="psum_s", bufs=2, space="PSUM") as pss:
                for c0 in range(0, HW, 512):
                    c1 = min(c0 + 512, HW)
                    ps = pss.tile([2, 512], F32, tag="pss")
                    nc.tensor.matmul(ps[:, 0:c1 - c0], mlpW[:, 320:322],
                                     hA[0:64, c0:c1], start=True, stop=True)
                    nc.vector.tensor_copy(s_rows[:, c0:c1], ps[:, 0:c1 - c0])

            # d = s0 - s1 (s1 via DMA to partition 0)
            nc.sync.dma_start(w1[:], s_rows[1:2, :])
            nc.vector.tensor_tensor(w2[:], s_rows[0:1, :], w1[:],
                                    op=ALU.subtract)
            # y = d + db;  softplus(y) = max(y,0) + ln(1+exp(-|y|))
            y = w2
            nc.vector.tensor_scalar_add(y[:], y[:], db[:, 0:1])
            nc.scalar.activation(w1[:], y[:], ACTF.Abs)
            nc.scalar.activation(w1[:], w1[:], ACTF.Exp, scale=-1.0)
            nc.scalar.activation(w1[:], w1[:], ACTF.Ln, bias=1.0)
            nc.vector.tensor_scalar_max(w3[:], y[:], 0.0)
            nc.vector.tensor_tensor(w3[:], w3[:], w1[:], op=ALU.add)  # sp
            # spN = -softplus(y); adj = spN - lse comes after the collective
            nc.vector.tensor_scalar_mul(spN[:, base:base + HW], w3[:], -1.0)
            if half == 0:
                # split0 = -softplus(-y) = y - softplus(y); split0E = exp
                nc.vector.tensor_tensor(s0E[:], y[:, 0:NF], w3[:, 0:NF],
                                        op=ALU.subtract)
                nc.scalar.activation(s0E[:], s0E[:], ACTF.Exp)

    # rule tables: softmax over 72 per res row
    rsum = keep.tile([36, 72], F32)
    rmax = keep.tile([36, 2], F32)
    rsumexp = keep.tile([36, 2], F32)
    nc.vector.tensor_tensor(rsum[:], ruleWb[:, 0:72], ruleWb[:, 72:144],
                            op=ALU.add)
    nc.vector.tensor_reduce(rmax[:, 0:1], rsum[:], axis=AXIS.X, op=ALU.max)
    nc.vector.tensor_scalar_mul(rmax[:, 1:2], rmax[:, 0:1], -1.0)
    nc.scalar.activation(rsum[:], rsum[:], ACTF.Exp, bias=rmax[:, 1:2],
                         accum_out=rsumexp[:, 0:1])
    nc.vector.reciprocal(rsumexp[:, 1:2], rsumexp[:, 0:1])
    nc.vector.tensor_scalar_mul(rsum[:], rsum[:], rsumexp[:, 1:2])

    # flatten ruleEn to [1, 2592] via DRAM, then fp16 G-flats (x 2^GBOOST)
    rule_d = dram.tile([36, 72], F32)
    nc.sync.dma_start(rule_d[:], rsum[:])
    nc.sync.dma_start(
        AP(ruleflat.tensor, ruleflat.offset,
           [[ruleflat.ap[0][0], 1], [1, 36 * 72]]),
        rule_d[:])
    g_d = dram.tile([2, 1296], F16)
    gtmp = keep.tile([1, 1296], F16)
    for row, off in ((0, 0), (1, 36)):   # 0: Gl (larg), 1: Gr (rarg)
        nc.vector.scalar_tensor_tensor(
            gtmp[:],
            mk(ruleflat, 1, off, [[72, 36], [1, 36]]),
            float(1 << GBOOST),
            mk(s0E, 1, 0, [[1, 36], [0, 36]]),
            op0=ALU.mult, op1=ALU.mult)
        nc.sync.dma_start(g_d[row:row + 1, :], gtmp[:])
    for dstt, row in ((glR, 0), (grR, 1)):
        nc.sync.dma_start(
            dstt[:],
            AP(g_d.tensor, g_d.offset + row * g_d.ap[0][0],
               [[0, 128], [1, 1296]]))
    nc.vector.tensor_copy(gfb[:, 0:16], mk(grR, 128, 0, [[36, 4], [1, 4]]))
    nc.vector.tensor_copy(gfb[:, 16:32], mk(glR, 128, 0, [[36, 4], [1, 4]]))

    # root: rsEn = softmax(root_W[0,0:4] + root_b[0:4]) replicated to 4 parts
    rs4 = keep.tile([1, 8], F32)
    rsE = keep.tile([1, 8], F32)
    rsEh = keep.tile([1, 4], F16)
    nc.vector.tensor_tensor(rs4[:, 0:4], smallv[:, 2:6], smallv[:, 6:10],
                            op=ALU.add)
    nc.vector.tensor_reduce(rs4[:, 4:5], rs4[:, 0:4], axis=AXIS.X, op=ALU.max)
    nc.vector.tensor_scalar_mul(rs4[:, 5:6], rs4[:, 4:5], -1.0)
    nc.scalar.activation(rsE[:, 0:4], rs4[:, 0:4], ACTF.Exp,
                         bias=rs4[:, 5:6], accum_out=rsE[:, 4:5])
    nc.vector.reciprocal(rsE[:, 5:6], rsE[:, 4:5])
    nc.vector.tensor_scalar_mul(rsE[:, 0:4], rsE[:, 0:4], rsE[:, 5:6])
    nc.vector.tensor_copy(rsEh[:], rsE[:, 0:4])
    rs_d = dram.tile([1, 4], F16)
    nc.sync.dma_start(rs_d[:], rsEh[:])
    nc.sync.dma_start(rsRep[:],
                      AP(rs_d.tensor, rs_d.offset, [[0, 4], [1, 4]]))

    # =======================================================================
    # Phase 3a (collective-independent): pb0 = wordW.T @ ntembT (no adj),
    # pow2 M1, beta1E' = 2^(pb0*log2e - M1i), raw W tables.
    # =======================================================================
    with tc.tile_pool(name="psum_b", bufs=1, space="PSUM") as psb:
        pb = psb.tile([PAIRS, CP], F32)
        for c0 in range(0, CP, 512):
            c1 = min(c0 + 512, CP)
            nc.tensor.matmul(pb[:, c0:c1], wordW[0:65, :], ntembT[:, c0:c1],
                             start=True, stop=True)
        # M1i = int(max(pb) * log2e); beta1E = 2^(pb*log2e - M1i)
        nc.vector.tensor_reduce(M1f[:, 2:3], pb[:, 0:C], axis=AXIS.X,
                                op=ALU.max)
        nc.vector.tensor_scalar(m1i[:, 0:1], M1f[:, 2:3], LOG2E, None,
                                op0=ALU.mult)                  # f32 -> i32
        nc.vector.tensor_scalar(M1f[:, 0:1], m1i[:, 0:1], 1.0, None,
                                op0=ALU.mult)                  # i32 -> f32
        nc.vector.tensor_scalar_mul(M1f[:, 1:2], M1f[:, 0:1], -LN2)
        nc.scalar.activation(beta1E[:], pb[:], ACTF.Exp,
                             bias=M1f[:, 1:2])

    # raw W tables [PAIRS, 1296] fp16 (adj applied after the collective)
    wblocks = [
        (0, [[36, 4], [1, 4]], "A", [[4, 4], [1, 4]]),
        (4, [[36, 4], [1, 32]], "B", [[32, 4], [1, 32]]),
        (144, [[1, 1152]], "C", [[1, 1152]]),
    ]
    for W, op_id, gR in ((WB, 0, grR), (WA, 1, glR)):
        off = lf_block_offsets(op_id)
        for (oo, od, key, idm) in wblocks:
            nc.vector.tensor_tensor(
                mk(W, PAIRS, oo, od),
                mk(beta1E, PAIRS, off[key], idm),
                mk(gR, PAIRS, oo, od),
                op=ALU.mult)

    # =======================================================================
    # Phase 3b (after the collective): lse, adjE2 = 2^(adj*log2e + K),
    # replicate, apply to W tables + lexical chart block 1.
    # =======================================================================
    nc.gpsimd.dma_start(sumexp_g[:], cc_out[:])
    nc.scalar.activation(lse21[:], sumexp_g[:], ACTF.Ln)
    # rearrange [128, NT] -> [1, CP]  (c = ct*128 + p) via DRAM bounce
    lse_d = dram.tile([128, NT], F32)
    nc.sync.dma_start(lse_d[:], lse21[:])
    nc.sync.dma_start(
        AP(lse_row.tensor, lse_row.offset,
           [[lse_row.ap[0][0], 1], [128, NT], [1, 128]]),
        AP(lse_d.tensor, lse_d.offset, [[lse_d.ap[0][0], 1], [1, NT], [NT, 128]]))
    nc.vector.tensor_tensor(adj[:], spN[:], lse_row[:], op=ALU.subtract)
    nc.vector.tensor_scalar(adj[:], adj[:], LOG2E, float(ADJK),
                            op0=ALU.mult, op1=ALU.add)
    adjE2 = p1.tile([1, CP], F16)
    nc.scalar.activation(adjE2[:], adj[:], ACTF.Exp, scale=LN2)
    adj_d = dram.tile([1, CP], F16)
    adjR = p1.tile([128, CP], F16)
    nc.sync.dma_start(adj_d[:], adjE2[:])
    nc.sync.dma_start(adjR[:],
                      AP(adj_d.tensor, adj_d.offset, [[0, 128], [1, CP]]))
    for W, op_id in ((WB, 0), (WA, 1)):
        off = lf_block_offsets(op_id)
        for (oo, od, key, idm) in wblocks:
            nc.vector.tensor_tensor(
                mk(W, PAIRS, oo, od),
                mk(W, PAIRS, oo, od),
                mk(adjR, PAIRS, off[key], idm),
                op=ALU.mult)

    # chart block 1 = beta1E' * adjE2; scale col 1 = M1i - K
    nc.vector.tensor_tensor(mk(chartA, PAIRS, BLK, [[1, 36]]),
                            beta1E[:, 0:NF], adjR[:, 0:NF], op=ALU.mult)
    nc.gpsimd.tensor_tensor(
        mk(chartA, PAIRS, BLK + 40, [[16, 2], [4, 4], [1, 4]]),
        mk(chartA, PAIRS, BLK + 4, [[16, 2], [4, 4], [1, 4]]),
        mk(gfb, PAIRS, 0, [[16, 2], [4, 4], [1, 4]]),
        op=ALU.mult)
    nc.vector.tensor_scalar_add(scaleA[:, 1:2], M1f[:, 0:1], float(-ADJK))
    # end-aligned block 1 is row-identical (end j = i+1 -> row (j-1)*4+b)
    nc.vector.tensor_copy(mk(chartE, PAIRS, (n - 1) * BLK, [[1, BLK]]),
                          mk(chartA, PAIRS, BLK, [[1, BLK]]))
    nc.vector.tensor_copy(scaleEr[:, n - 1:n], scaleA[:, 1:2])

    ph1.close()  # free ph1 tensors before the CKY working set

    es2 = contextlib.ExitStack()
    stage_pool = es2.enter_context(tc.tile_pool(name="stage", bufs=2))
    stageS_pool = es2.enter_context(tc.tile_pool(name="stageS", bufs=2))
    wash_pool = es2.enter_context(tc.tile_pool(name="wash", bufs=2))
    scr = es2.enter_context(tc.tile_pool(name="cky", bufs=2))
    scr1 = es2.enter_context(tc.tile_pool(name="cky1", bufs=1))

    # =======================================================================
    # Phase 4: CKY in scaled-exp space, fp16 data, integer pow2 scales.
    # chartA block L cols [L*BLK .. +72]: [0:36 chart | 40:56 FB | 56:72 FA]
    # chartE: same block layout at col (n-m)*BLK, rows indexed by span end.
    # scaleA col L / scaleEr col (n-m): integer scales as fp32.
    # true = stored * 2^scale;  G tables carry 2^GBOOST.
    # =======================================================================
    for L in range(2, n + 1):
        S = n - L + 1
        PS = 4 * S
        PSf = 4 * (S + 1)   # rows of the fresh block L-1
        NI = L - 2

        stageH = stage_pool.tile([128, (n + 1) * BLK], F16, tag="st")
        stageS = stageS_pool.tile([128, n + 2], F32, tag="sts")
        wash = wash_pool.tile([128, 1296], F16, tag="wa")

        # -- critical: fresh block L-1 (rows i+1 -> i), + its scale col.
        # chart cols [0:36] land first (unblock prodB); FA cols follow.
        nc.sync.dma_start(
            mk(stageH, PS, (n - L + 1) * BLK, [[1, 36]]),
            mk(chartA, PS, (L - 1) * BLK, [[1, 36]], base_part=4))
        nc.sync.dma_start(
            mk(stageS, PS, n - L + 1, [[1, 1]]),
            mk(scaleA, PS, L - 1, [[1, 1]], base_part=4))
        if NI > 0:
            nc.sync.dma_start(
                mk(stageH, PS, (n - L + 1) * BLK + 56, [[1, 16]]),
                mk(chartA, PS, (L - 1) * BLK + 56, [[1, 16]], base_part=4))
        # -- writeback fresh block L-1 to end-aligned history
        if L + 1 <= n:
            nc.sync.dma_start(
                mk(chartE, PSf, (n - L + 1) * BLK, [[1, BLK]],
                   base_part=4 * (L - 2)),
                mk(chartA, PSf, (L - 1) * BLK, [[1, BLK]]))
            nc.sync.dma_start(
                mk(scaleEr, PSf, n - L + 1, [[1, 1]], base_part=4 * (L - 2)),
                mk(scaleA, PSf, L - 1, [[1, 1]]))
        # -- prefetchable history: blocks 1..L-2 (+ scales)
        if L >= 3:
            nc.sync.dma_start(
                mk(stageH, PS, (n - L + 2) * BLK, [[1, (L - 2) * BLK]]),
                mk(chartE, PS, (n - L + 2) * BLK, [[1, (L - 2) * BLK]],
                   base_part=4 * (L - 1)))
            nc.sync.dma_start(
                mk(stageS, PS, n - L + 2, [[1, L - 2]]),
                mk(scaleEr, PS, n - L + 2, [[1, L - 2]],
                   base_part=4 * (L - 1)))
        # -- wash: WA rows at the right-end pair
        nc.sync.dma_start(
            mk(wash, PS, 0, [[1, 1296]]),
            mk(WA, PS, 0, [[1, 1296]], base_part=4 * (L - 1)))

        # ---- edge-A products (free of the critical DMAs)
        prodAB = scr1.tile([128, 2592], F16, tag="prod")
        nc.vector.tensor_tensor(
            prodAB[0:PS, 0:1296],
            mk(wash, PS, 0, [[1, 1296]]),
            mk(chartA, PS, (L - 1) * BLK, [[0, 36], [1, 36]]),
            op=ALU.mult)

        # ---- edge-B products (right after the fresh chart DMA lands)
        nc.vector.tensor_tensor(
            prodAB[0:PS, 1296:2592],
            mk(WB, PS, 0, [[1, 1296]]),
            mk(stageH, PS, (n - L + 1) * BLK, [[0, 36], [1, 36]]),
            op=ALU.mult)
        red72 = scr.tile([128, 160], F16, tag="red")
        nc.vector.tensor_reduce(red72[0:PS, 0:36],
                                mk(prodAB, PS, 0, [[36, 36], [1, 36]]),
                                axis=AXIS.X, op=ALU.add)
        nc.vector.tensor_reduce(red72[0:PS, 36:72],
                                mk(prodAB, PS, 1296, [[36, 36], [1, 36]]),
                                axis=AXIS.X, op=ALU.add)

        # ---- scales: sAsm[k] = sI_k, k=1..L-1; note the edge cases reuse
        # interior scales: sA = sI_{L-1} (k=L-1 split), sB = sI_1.
        sAsm = scr.tile([128, n + 8], F32, tag="sasm")
        eBits = scr.tile([128, n + 8], I32, tag="ebits")
        mstarN = scr.tile([128, 2], F32, tag="mstar")
        nc.vector.tensor_tensor(
            sAsm[0:PS, 1:L],
            scaleA[0:PS, 1:L],
            stageS[0:PS, n - L + 1:n], op=ALU.add)
        nc.vector.tensor_reduce(mstarN[0:PS, 0:1], sAsm[0:PS, 1:L],
                                axis=AXIS.X, op=ALU.max, negate=True)
        nc.vector.tensor_scalar(sAsm[0:PS, 1:L], sAsm[0:PS, 1:L],
                                mstarN[0:PS, 0:1], -126.0,
                                op0=ALU.add, op1=ALU.max)
        nc.vector.tensor_scalar(eBits[0:PS, 1:L], sAsm[0:PS, 1:L],
                                127.0, 8388608.0, op0=ALU.add, op1=ALU.mult)
        eAll = lambda off, dims: mk(eBits, PS, off, dims).bitcast(F32)

        # ---- interior terms (res<4), batched over k, on GpSimd
        if NI > 0:
            tI = scr1.tile([128, 2 * max(n - 2, 1) * 16], F16, tag="ti")
            nc.gpsimd.tensor_tensor(   # IA: chart[k][i] args x stage FA(L-k)
                mk(tI, PS, 0, [[2 * NI * 4, 4], [4, NI], [1, 4]]),
                mk(chartA, PS, BLK, [[0, 4], [BLK, NI], [1, 4]]),
                mk(stageH, PS, (n - L + 1) * BLK + 56,
                   [[4, 4], [BLK, NI], [1, 4]]),
                op=ALU.mult)
            nc.gpsimd.tensor_tensor(   # IB: stage args(L-k) x chart[k] FB
                mk(tI, PS, NI * 4, [[2 * NI * 4, 4], [4, NI], [1, 4]]),
                mk(stageH, PS, (n - L + 2) * BLK, [[0, 4], [BLK, NI], [1, 4]]),
                mk(chartA, PS, 2 * BLK + 40, [[4, 4], [BLK, NI], [1, 4]]),
                op=ALU.mult)
            for half in range(2):      # x eI (k scales), in place
                nc.gpsimd.tensor_tensor(
                    mk(tI, PS, half * NI * 4,
                       [[2 * NI * 4, 4], [4, NI], [1, 4]]),
                    mk(tI, PS, half * NI * 4,
                       [[2 * NI * 4, 4], [4, NI], [1, 4]]),
                    eAll(1 + half, [[0, 4], [1, NI], [0, 4]]),
                    op=ALU.mult)
            nc.gpsimd.tensor_tensor(   # fold IB half into IA half
                mk(tI, PS, 0, [[2 * NI * 4, 4], [4, NI], [1, 4]]),
                mk(tI, PS, 0, [[2 * NI * 4, 4], [4, NI], [1, 4]]),
                mk(tI, PS, NI * 4, [[2 * NI * 4, 4], [4, NI], [1, 4]]),
                op=ALU.add)

        # ---- combine cases with their pow2 scale factors (eA at col L-1,
        # eB at col 1: negative-stride pair view)
        nc.vector.tensor_tensor(red72[0:PS, 72:144], red72[0:PS, 0:72],
                                eAll(L - 1, [[-(L - 2), 2], [0, 36]]),
                                op=ALU.mult)
        tot36 = scr.tile([128, 40], F16, tag="tot")
        nc.vector.tensor_tensor(tot36[0:PS, 0:36], red72[0:PS, 72:108],
                                red72[0:PS, 108:144], op=ALU.add)
        if NI > 0:
            nc.vector.tensor_reduce(   # sum over (k, arg) -> [PS, 4]
                tot36[0:PS, 36:40],
                mk(tI, PS, 0,
                   [[2 * NI * 4, 4], [4, NI], [1, 4]]),
                axis=AXIS.XY, op=ALU.add)
            nc.vector.tensor_tensor(tot36[0:PS, 0:4], tot36[0:PS, 0:4],
                                    tot36[0:PS, 36:40], op=ALU.add)

        # ---- pow2 renorm and write chart block L
        # scale carries a +(127+GBOOST) per-level offset, removed at the end
        mvE = scr.tile([128, 4], F32, tag="mval")
        mvI = lambda c: mvE[0:PS, c:c + 1].bitcast(I32)
        nc.vector.tensor_reduce(mvE[0:PS, 0:1], tot36[0:PS, 0:36],
                                axis=AXIS.X, op=ALU.max)
        nc.vector.tensor_scalar(mvI(2), mvI(0), expMask[0:PS, 0:1], None,
                                op0=ALU.bitwise_and)
        nc.vector.reciprocal(mvE[0:PS, 1:2], mvE[0:PS, 2:3])
        nc.vector.tensor_scalar(
            chartA[0:PS, L * BLK:L * BLK + 36],
            tot36[0:PS, 0:36], mvE[0:PS, 1:2], None, op0=ALU.mult)
        nc.vector.tensor_scalar(scaleA[0:PS, L:L + 1], mvI(2),
                                1.0 / 8388608.0, mstarN[0:PS, 0:1],
                                op0=ALU.mult, op1=ALU.subtract)
        nc.gpsimd.tensor_tensor(
            mk(chartA, PS, L * BLK + 40, [[16, 2], [4, 4], [1, 4]]),
            mk(chartA, PS, L * BLK + 4, [[16, 2], [4, 4], [1, 4]]),
            mk(gfb, PS, 0, [[16, 2], [4, 4], [1, 4]]),
            op=ALU.mult)

    # =======================================================================
    # Phase 5: root -> nll per sentence
    # =======================================================================
    nc.vector.tensor_tensor(finh[:, 0:4],
                            mk(chartA, 4, n * BLK, [[1, 4]]),
                            rsRep[:], op=ALU.mult)
    nc.vector.tensor_reduce(fin[:, 4:5], finh[:, 0:4], axis=AXIS.X,
                            op=ALU.add)
    nc.scalar.activation(fin[:, 5:6], fin[:, 4:5], ACTF.Ln)
    # nll = -(ln(fin) + scale * ln2); scale carries +(127+GBOOST)(n-1)
    nc.vector.scalar_tensor_tensor(
        fin[:, 6:7], scaleA[0:4, n:n + 1], -LN2,
        fin[:, 5:6], op0=ALU.mult, op1=ALU.subtract)
    nc.vector.tensor_scalar_add(fin[:, 7:8], fin[:, 6:7],
                                float((127 + GBOOST) * (n - 1)) * LN2)
    nc.sync.dma_start(d["out"][:], fin[:, 7:8])
    es2.close()
    es.close()


# ============================================================== host wrapper
_PROG_CACHE = {}


def _get_program(cfg: Cfg):
    key = (cfg.n, cfg.v_loc, cfg.n_cores)
    if key not in _PROG_CACHE:
        _PROG_CACHE[key] = build_program(cfg)
    return _PROG_CACHE[key]


def make_inmaps(cfg: Cfg, inputs):
    """Host-side shard/pack of FULL inputs -> per-core DRAM input dicts."""
    import ml_dtypes
    bf16 = ml_dtypes.bfloat16
    x = np.asarray(inputs["x"])
    check_functor_tables(np.asarray(inputs["l_functors"]),
                         np.asarray(inputs["r_functors"]))
    nt_emb = np.asarray(inputs["nt_emb"], np.float32)          # [C, D]
    vocab_W = np.asarray(inputs["vocab_W"], np.float32)        # [D, V]
    vocab_b = np.asarray(inputs["vocab_b"], np.float32)        # [V]

    ntembT = np.zeros((65, CP), bf16)
    ntembT[0:64, 0:C] = nt_emb.T.astype(bf16)
    ntembT[64, :] = 1.0

    mlpW = np.zeros((64, 322), bf16)
    for j, k in enumerate(("sW1", "r1W1", "r1W2", "r2W1", "r2W2")):
        mlpW[:, j * 64:(j + 1) * 64] = np.asarray(inputs[k], np.float32)
    mlpW[:, 320:322] = np.asarray(inputs["sW2"], np.float32)

    mlpB = np.zeros((64, 8), np.float32)
    for j, k in enumerate(("sb1", "r1b1", "r1b2", "r2b1", "r2b2")):
        mlpB[:, j] = np.asarray(inputs[k], np.float32)

    ruleWb = np.zeros((36, 144), np.float32)
    ruleWb[:, 0:72] = np.asarray(inputs["rule_W"], np.float32)
    ruleWb[:, 72:144] = np.tile(
        np.asarray(inputs["rule_b"], np.float32)[None, :], (36, 1))

    smallv = np.zeros((1, 16), np.float32)
    smallv[0, 0:2] = np.asarray(inputs["sb2"], np.float32)
    smallv[0, 2:6] = np.asarray(inputs["root_W"], np.float32)[0, 0:4]
    smallv[0, 6:10] = np.asarray(inputs["root_b"], np.float32)[0:4]

    vs = cfg.v_loc
    in_maps = []
    for core in range(cfg.n_cores):
        vocabW = np.zeros((65, cfg.v_pad), bf16)
        vocabW[64, :] = NEGB
        vocabW[0:64, 0:vs] = vocab_W[:, core * vs:(core + 1) * vs]
        vocabW[64, 0:vs] = vocab_b[core * vs:(core + 1) * vs]

        words = x[core * BLOC:(core + 1) * BLOC, 0:cfg.n]   # [BLOC, n]
        wid = words.T.reshape(-1)                           # pair = i*4 + b
        wordW = np.zeros((66, cfg.pairs), bf16)
        wordW[0:64, :] = vocab_W[:, wid].astype(bf16)
        wordW[64, :] = vocab_b[wid]
        wordW[65, :] = 1.0

        in_maps.append({
            "ntembT": ntembT, "vocabW": vocabW, "wordW": wordW,
            "mlpW": mlpW, "mlpB": mlpB, "ruleWb": ruleWb, "smallv": smallv,
        })
    return in_maps


def kernel(**inputs) -> np.ndarray:
    cfg = Cfg(n=32, v_loc=V // NCORES, n_cores=NCORES)
    nc = _get_program(cfg)
    in_maps = make_inmaps(cfg, inputs)
    res = bass_utils.run_bass_kernel_spmd(
        nc, in_maps, core_ids=list(range(cfg.n_cores)))
    out = np.concatenate([r["out_nll"].reshape(-1) for r in res.results])
    return out.astype(np.float32)


if __name__ == "__main__":
    from reference import setup_inputs, reference
    inputs = {k: np.asarray(v) for k, v in setup_inputs().items()}
    got = kernel(**inputs)
    exp = np.asarray(reference(**inputs))
    rel = np.max(np.abs(got - exp) / np.maximum(np.abs(exp), 1e-6))
    print("expected:", exp[:8])
    print("got     :", got[:8])
    print("Relative error:", rel)


# revision 65
# speedup vs baseline: 1.4211x; 1.4211x over previous
"""Trainium2 Bass kernel for nn_BasicCGInducer (CKY inside algorithm for a
categorial-grammar inducer).

Strategy (8 NeuronCores):
  - Data-parallel over sentences: core j handles sentences 4j..4j+3.
  - Emission log-partition ([C,V] softmax denominator) is tensor-parallel
    over vocab: each core exps a 4000-column V-shard, one AllReduce of [C].
  - CKY inside pass runs per-core in scaled-exp space with POWER-OF-TWO
    integer span scales: the per-level rescales are pure DVE bit/int ops
    (no activation-table Exp/Ln in the loop), and all chart data, rule
    tables and products are fp16 to engage the DVE 2x/4x perf modes.
  - All matmuls (emission, split-MLP, beta1) run in bf16.

kernel(**inputs) takes FULL inputs, shards on host, runs one SPMD bass
program on cores 0-7, and reassembles the [32] output.
"""
import sys
import contextlib

sys.path.insert(0, "/opt/trn_rl_repo")

import numpy as np

import concourse.bass as bass
import concourse.bacc as bacc
import concourse.mybir as mybir
import concourse.tile as tile
from concourse.ap import AP
from concourse import bass_utils

F32 = mybir.dt.float32
F16 = mybir.dt.float16
BF16 = mybir.dt.bfloat16
I32 = mybir.dt.int32
ALU = mybir.AluOpType
ACTF = mybir.ActivationFunctionType
AXIS = mybir.AxisListType

# ---------------------------------------------------------------- constants
P4 = 4          # primitive cats
NF = 36         # non-functor cats
C = 2596        # total cats
CP = 2688       # padded C (21 * 128)
NT = CP // 128  # 21 c-tiles
D = 64
B = 32          # total sentences
NCORES = 8
BLOC = B // NCORES  # 4 sentences per core
V = 32000
BLK = 72        # chart block: [0:36 chart | 36:40 pad | 40:56 FB | 56:72 FA]
NEGB = -30.0    # bias for padded vocab columns (exp(-30) ~ 0 vs Z ~ 6e3)
GBOOST = 10     # G tables carry 2^GBOOST; span scale compensates
LN2 = 0.6931471805599453
LOG2E = 1.4426950408889634
# Schraudolph fast-exp: bitcast_f32(int32(x * 2^23/ln2 + b)), mean-unbiased b
SCH_A = 12102203.161561485
SCH_B = 1064866805.0
ADJK = 16       # 2^ADJK boost on the emission adj factor (lse ~ 10.9 nats)


class Cfg:
    def __init__(self, n=32, v_loc=4000, n_cores=8):
        self.n = n                      # sentence length
        self.v_loc = v_loc              # vocab shard per core
        self.v_pad = ((v_loc + 511) // 512) * 512
        self.n_cores = n_cores
        self.pairs = 4 * n              # (i, b) pairs on partitions


# ------------------------------------------------------------ functor maps
def lf_block_offsets(op):
    """c = off + {A: 4r+a | B: 32r+(a-4) | C: 36(r-4)+a} per derivation of
    the deterministic functor-id tables. op=0 -> l_functors, 1 -> r_functors."""
    return {
        "A": 4 + 16 * op,            # res<4, arg<4 : c = A + 4*res + arg
        "B": 36 + 1280 * op,         # res<4, arg>=4: c = B + 32*res + (arg-4)
        "C": 164 + 1280 * op,        # res>=4      : c = C0 + 36*(res-4) + arg
    }


def check_functor_tables(l_functors, r_functors):
    for op, tab in ((0, l_functors), (1, r_functors)):
        off = lf_block_offsets(op)
        exp = np.zeros((NF, NF), np.int64)  # [arg, res]
        for res in range(NF):
            for arg in range(NF):
                if res < P4 and arg < P4:
                    exp[arg, res] = off["A"] + 4 * res + arg
                elif res < P4:
                    exp[arg, res] = off["B"] + 32 * res + (arg - 4)
                else:
                    exp[arg, res] = off["C"] + 36 * (res - 4) + arg
        assert np.array_equal(np.asarray(tab, np.int64), exp), (
            f"functor table structure mismatch (op={op})")


# ---------------------------------------------------------------- AP helper
def mk(t, parts, off, dims, base_part=0):
    """Raw AP on tile t: partition range [base_part, base_part+parts),
    free offset `off` (elements), extra free dims [[step, count], ...]."""
    w = t.ap[0][0]
    return AP(t.tensor, t.offset + base_part * w + off, [[w, parts]] + dims)


# ============================================================ device program
def build_program(cfg: Cfg):
    nc = bacc.Bacc("TRN2", target_bir_lowering=False, debug=False,
                   num_devices=cfg.n_cores)
    d = {
        "ntembT": nc.dram_tensor("ntembT", [65, CP], BF16,
                                 kind="ExternalInput"),
        "vocabW": nc.dram_tensor("vocabW", [65, cfg.v_pad], BF16,
                                 kind="ExternalInput"),
        "wordW": nc.dram_tensor("wordW", [66, cfg.pairs], BF16,
                                kind="ExternalInput"),
        "mlpW": nc.dram_tensor("mlpW", [64, 322], BF16, kind="ExternalInput"),
        "mlpB": nc.dram_tensor("mlpB", [64, 8], F32, kind="ExternalInput"),
        "ruleWb": nc.dram_tensor("ruleWb", [36, 144], F32,
                                 kind="ExternalInput"),
        "smallv": nc.dram_tensor("smallv", [1, 16], F32,
                                 kind="ExternalInput"),
        "out": nc.dram_tensor("out_nll", [BLOC, 1], F32,
                              kind="ExternalOutput"),
    }
    with tile.TileContext(nc) as tc:
        with nc.allow_low_precision(reason="fp16 CKY inside pass"):
            _trace(tc, cfg, d)
    nc.compile()
    return nc


def _trace(tc, cfg, d):
    nc = tc.nc
    n, PAIRS, VP = cfg.n, cfg.pairs, cfg.v_pad
    NV = VP // 512                    # 512-col v-tiles per core
    NHALF = (NV + 3) // 4             # ACT chunks of up to 4 v-tiles
    HW = CP // 2                      # MLP half width (1344)

    es = contextlib.ExitStack()
    keep = es.enter_context(tc.tile_pool(name="keep", bufs=1))
    dram = es.enter_context(tc.tile_pool(name="dram", bufs=1, space="DRAM"))

    # ---------------- long-lived tensors
    chartA = keep.tile([PAIRS, (n + 1) * BLK], F16)
    chartE = keep.tile([PAIRS, (n + 1) * BLK], F16)
    scaleA = keep.tile([PAIRS, n + 2], F32)   # col L = scale of chart[L]
    scaleEr = keep.tile([PAIRS, n + 2], F32)  # col n-m = scale of end block m
    WA = keep.tile([PAIRS, 1296], F16)
    WB = keep.tile([PAIRS, 1296], F16)
    glR = keep.tile([128, 1296], F16)
    grR = keep.tile([128, 1296], F16)
    gfb = keep.tile([128, 32], F16)           # [0:16 grR 4x4 | 16:32 glR 4x4]
    M1f = keep.tile([PAIRS, 4], F32)          # 0: M1i (int-val), 1: -ln2*M1i
    m1i = keep.tile([PAIRS, 2], I32)
    mlpB = keep.tile([64, 8], F32)
    smallv = keep.tile([1, 16], F32)
    sumexp_parts = keep.tile([128, NT * NHALF], F32)
    sumexp_loc = keep.tile([128, NT], F32)
    sumexp_g = keep.tile([128, NT], F32)
    lse21 = keep.tile([128, NT], F32)
    s0E = keep.tile([1, NF], F32)
    db = keep.tile([1, 2], F32)
    rsRep = keep.tile([4, 4], F16)
    fin = keep.tile([4, 8], F32)
    finh = keep.tile([4, 4], F16)
    expMask = keep.tile([128, 1], I32)
    nc.gpsimd.memset(expMask[:], 0x7F800000)

    nc.sync.dma_start(mlpB[:], d["mlpB"][:])
    nc.sync.dma_start(smallv[:], d["smallv"][:])
    nc.gpsimd.memset(chartA[:], 0.0)
    nc.gpsimd.memset(chartE[:], 0.0)
    nc.gpsimd.memset(scaleA[:], 0.0)
    nc.gpsimd.memset(scaleEr[:], 0.0)

    ph1 = contextlib.ExitStack()
    p1 = ph1.enter_context(tc.tile_pool(name="ph1", bufs=1))
    ntembT = p1.tile([65, CP], BF16)
    vocabW = p1.tile([65, VP], BF16)
    wordW = p1.tile([66, PAIRS], BF16)
    mlpW = p1.tile([64, 322], BF16)
    ruleWb = p1.tile([36, 144], F32)
    lse_row = p1.tile([1, CP], F32)
    adj = p1.tile([1, CP], F32)
    spN = p1.tile([1, CP], F32)
    beta1E = p1.tile([PAIRS, CP], F16)
    ruleflat = p1.tile([1, 36 * 72], F32)

    nc.sync.dma_start(ntembT[:], d["ntembT"][:])
    nc.sync.dma_start(vocabW[:], d["vocabW"][:])
    nc.sync.dma_start(wordW[:], d["wordW"][:])
    nc.sync.dma_start(mlpW[:], d["mlpW"][:])
    nc.sync.dma_start(ruleWb[:], d["ruleWb"][:])

    # =======================================================================
    # Phase 1: emission partition function. exp chunks split between the
    # scalar engine (table exp + accum) and DVE (Schraudolph bitcast exp).
    # =======================================================================
    with tc.tile_pool(name="psum_e", bufs=2, space="PSUM") as pse, \
         tc.tile_pool(name="scr_e", bufs=2) as scre:
        for ct in range(NT):
            for h in range(NHALF):
                vt0 = h * 4
                nvt = min(4, NV - vt0)
                pt = pse.tile([128, 512 * nvt], F32, tag="pse")
                for vt in range(nvt):
                    nc.tensor.matmul(
                        pt[:, vt * 512:(vt + 1) * 512],
                        ntembT[:, ct * 128:(ct + 1) * 128],
                        vocabW[:, (vt0 + vt) * 512:(vt0 + vt + 1) * 512],
                        start=True, stop=True)
                acc = sumexp_parts[:, ct * NHALF + h:ct * NHALF + h + 1]
                ci = ct * NHALF + h
                if (ci * 15) % 42 >= 15:  # 27: scalar-engine exp + accum
                    sce = scre.tile([128, 512 * 4], F32, tag="scre")
                    nc.scalar.activation(
                        sce[:, 0:512 * nvt], pt[:], ACTF.Exp, accum_out=acc)
                else:                 # 14: DVE Schraudolph exp + sum
                    sci = scre.tile([128, 512 * 4], I32, tag="scri")
                    nc.vector.tensor_scalar(
                        sci[:, 0:512 * nvt], pt[:], SCH_A, SCH_B,
                        op0=ALU.mult, op1=ALU.add)
                    nc.vector.tensor_reduce(
                        acc, sci[:, 0:512 * nvt].bitcast(F32),
                        axis=AXIS.X, op=ALU.add)

    if NHALF > 1:
        nc.vector.tensor_reduce(
            sumexp_loc[:],
            mk(sumexp_parts, 128, 0, [[NHALF, NT], [1, NHALF]]),
            axis=AXIS.X, op=ALU.add)
    else:
        nc.vector.tensor_copy(sumexp_loc[:], sumexp_parts[:, 0:NT])

    # AllReduce over cores via DRAM bounce
    cc_in = dram.tile([128, NT], F32)
    cc_out = dram.tile([128, NT], F32)
    nc.sync.dma_start(cc_in[:], sumexp_loc[:])
    nc.gpsimd.collective_compute(
        "AllReduce", ALU.add,
        replica_groups=[list(range(cfg.n_cores))],
        ins=[cc_in[:].opt()], outs=[cc_out[:].opt()])
    # the collective-dependent lse/adj tail is issued after phase 3's
    # collective-independent work so no engine queue blocks on it.

    # =======================================================================
    # Phase 2: split MLP (transposed layout hT [64, *]), rule tables, root
    # =======================================================================
    nc.vector.tensor_tensor(db[:, 0:1], smallv[:, 0:1], smallv[:, 1:2],
                            op=ALU.subtract)
    nc.vector.tensor_scalar_mul(db[:, 1:2], db[:, 0:1], -1.0)

    with tc.tile_pool(name="mlp", bufs=1) as mlp:
        hA = mlp.tile([64, HW], BF16, tag="hA")
        hB = mlp.tile([64, HW], BF16, tag="hB")
        hC = mlp.tile([64, HW], BF16, tag="hC")
        s_rows = mlp.tile([2, HW], F32, tag="srows")
        w1 = mlp.tile([1, HW], F32, tag="w1")
        w2 = mlp.tile([1, HW], F32, tag="w2")
        w3 = mlp.tile([1, HW], F32, tag="w3")

        for half in range(2):
            base = half * HW

            def dense_relu(dst, col0, rhs, bias_col, res_add=None, rb=0,
                           relu=True):
                with tc.tile_pool(name="psum_m", bufs=2,
                                  space="PSUM") as psm:
                    for c0 in range(0, HW, 512):
                        c1 = min(c0 + 512, HW)
                        pm = psm.tile([64, 512], F32, tag="psm")
                        nc.tensor.matmul(pm[:, 0:c1 - c0],
                                         mlpW[:, col0:col0 + 64],
                                         rhs[0:64, rb + c0:rb + c1],
                                         start=True, stop=True)
                        nc.scalar.activation(
                            dst[:, c0:c1], pm[:, 0:c1 - c0],
                            ACTF.Relu if relu else ACTF.Identity,
                            bias=mlpB[0:64, bias_col:bias_col + 1])
                        if res_add is not None:
                            nc.vector.tensor_tensor(
                                dst[:, c0:c1], dst[:, c0:c1],
                                res_add[:, c0:c1], op=ALU.add)

            dense_relu(hA, 0, ntembT, 0, rb=base,
                       relu=False)                      # h1 (linear)
            dense_relu(hB, 64, hA, 1)                   # t = relu(h1 W + b)
            dense_relu(hC, 128, hB, 2, res_add=hA)      # h2
            dense_relu(hB, 192, hC, 3)                  # t2
            dense_relu(hA, 256, hB, 4, res_add=hC)      # h3

            with tc.tile_pool(name="psum_s", bufs=2, space="PSUM") as pss:
                for c0 in range(0, HW, 512):
                    c1 = min(c0 + 512, HW)
                    ps = pss.tile([2, 512], F32, tag="pss")
                    nc.tensor.matmul(ps[:, 0:c1 - c0], mlpW[:, 320:322],
                                     hA[0:64, c0:c1], start=True, stop=True)
                    nc.vector.tensor_copy(s_rows[:, c0:c1], ps[:, 0:c1 - c0])

            # d = s0 - s1 (s1 via DMA to partition 0)
            nc.sync.dma_start(w1[:], s_rows[1:2, :])
            nc.vector.tensor_tensor(w2[:], s_rows[0:1, :], w1[:],
                                    op=ALU.subtract)
            # y = d + db;  softplus(y) = max(y,0) + ln(1+exp(-|y|))
            y = w2
            nc.vector.tensor_scalar_add(y[:], y[:], db[:, 0:1])
            nc.scalar.activation(w1[:], y[:], ACTF.Abs)
            nc.scalar.activation(w1[:], w1[:], ACTF.Exp, scale=-1.0)
            nc.scalar.activation(w1[:], w1[:], ACTF.Ln, bias=1.0)
            nc.vector.tensor_scalar_max(w3[:], y[:], 0.0)
            nc.vector.tensor_tensor(w3[:], w3[:], w1[:], op=ALU.add)  # sp
            # spN = -softplus(y); adj = spN - lse comes after the collective
            nc.vector.tensor_scalar_mul(spN[:, base:base + HW], w3[:], -1.0)
            if half == 0:
                # split0 = -softplus(-y) = y - softplus(y); split0E = exp
                nc.vector.tensor_tensor(s0E[:], y[:, 0:NF], w3[:, 0:NF],
                                        op=ALU.subtract)
                nc.scalar.activation(s0E[:], s0E[:], ACTF.Exp)

    # rule tables: softmax over 72 per res row
    rsum = keep.tile([36, 72], F32)
    rmax = keep.tile([36, 2], F32)
    rsumexp = keep.tile([36, 2], F32)
    nc.vector.tensor_tensor(rsum[:], ruleWb[:, 0:72], ruleWb[:, 72:144],
                            op=ALU.add)
    nc.vector.tensor_reduce(rmax[:, 0:1], rsum[:], axis=AXIS.X, op=ALU.max)
    nc.vector.tensor_scalar_mul(rmax[:, 1:2], rmax[:, 0:1], -1.0)
    nc.scalar.activation(rsum[:], rsum[:], ACTF.Exp, bias=rmax[:, 1:2],
                         accum_out=rsumexp[:, 0:1])
    nc.vector.reciprocal(rsumexp[:, 1:2], rsumexp[:, 0:1])
    nc.vector.tensor_scalar_mul(rsum[:], rsum[:], rsumexp[:, 1:2])

    # flatten ruleEn to [1, 2592] via DRAM, then fp16 G-flats (x 2^GBOOST)
    rule_d = dram.tile([36, 72], F32)
    nc.sync.dma_start(rule_d[:], rsum[:])
    nc.sync.dma_start(
        AP(ruleflat.tensor, ruleflat.offset,
           [[ruleflat.ap[0][0], 1], [1, 36 * 72]]),
        rule_d[:])
    g_d = dram.tile([2, 1296], F16)
    gtmp = keep.tile([1, 1296], F16)
    for row, off in ((0, 0), (1, 36)):   # 0: Gl (larg), 1: Gr (rarg)
        nc.vector.scalar_tensor_tensor(
            gtmp[:],
            mk(ruleflat, 1, off, [[72, 36], [1, 36]]),
            float(1 << GBOOST),
            mk(s0E, 1, 0, [[1, 36], [0, 36]]),
            op0=ALU.mult, op1=ALU.mult)
        nc.sync.dma_start(g_d[row:row + 1, :], gtmp[:])
    for dstt, row in ((glR, 0), (grR, 1)):
        nc.sync.dma_start(
            dstt[:],
            AP(g_d.tensor, g_d.offset + row * g_d.ap[0][0],
               [[0, 128], [1, 1296]]))
    nc.vector.tensor_copy(gfb[:, 0:16], mk(grR, 128, 0, [[36, 4], [1, 4]]))
    nc.vector.tensor_copy(gfb[:, 16:32], mk(glR, 128, 0, [[36, 4], [1, 4]]))

    # root: rsEn = softmax(root_W[0,0:4] + root_b[0:4]) replicated to 4 parts
    rs4 = keep.tile([1, 8], F32)
    rsE = keep.tile([1, 8], F32)
    rsEh = keep.tile([1, 4], F16)
    nc.vector.tensor_tensor(rs4[:, 0:4], smallv[:, 2:6], smallv[:, 6:10],
                            op=ALU.add)
    nc.vector.tensor_reduce(rs4[:, 4:5], rs4[:, 0:4], axis=AXIS.X, op=ALU.max)
    nc.vector.tensor_scalar_mul(rs4[:, 5:6], rs4[:, 4:5], -1.0)
    nc.scalar.activation(rsE[:, 0:4], rs4[:, 0:4], ACTF.Exp,
                         bias=rs4[:, 5:6], accum_out=rsE[:, 4:5])
    nc.vector.reciprocal(rsE[:, 5:6], rsE[:, 4:5])
    nc.vector.tensor_scalar_mul(rsE[:, 0:4], rsE[:, 0:4], rsE[:, 5:6])
    nc.vector.tensor_copy(rsEh[:], rsE[:, 0:4])
    rs_d = dram.tile([1, 4], F16)
    nc.sync.dma_start(rs_d[:], rsEh[:])
    nc.sync.dma_start(rsRep[:],
                      AP(rs_d.tensor, rs_d.offset, [[0, 4], [1, 4]]))

    # =======================================================================
    # Phase 3a (collective-independent): pb0 = wordW.T @ ntembT (no adj),
    # pow2 M1, beta1E' = 2^(pb0*log2e - M1i), raw W tables.
    # =======================================================================
    with tc.tile_pool(name="psum_b", bufs=1, space="PSUM") as psb:
        pb = psb.tile([PAIRS, CP], F32)
        for c0 in range(0, CP, 512):
            c1 = min(c0 + 512, CP)
            nc.tensor.matmul(pb[:, c0:c1], wordW[0:65, :], ntembT[:, c0:c1],
                             start=True, stop=True)
        # M1i = int(max(pb) * log2e); beta1E = 2^(pb*log2e - M1i)
        nc.vector.tensor_reduce(M1f[:, 2:3], pb[:, 0:C], axis=AXIS.X,
                                op=ALU.max)
        nc.vector.tensor_scalar(m1i[:, 0:1], M1f[:, 2:3], LOG2E, None,
                                op0=ALU.mult)                  # f32 -> i32
        nc.vector.tensor_scalar(M1f[:, 0:1], m1i[:, 0:1], 1.0, None,
                                op0=ALU.mult)                  # i32 -> f32
        nc.vector.tensor_scalar_mul(M1f[:, 1:2], M1f[:, 0:1], -LN2)
        nc.scalar.activation(beta1E[:], pb[:], ACTF.Exp,
                             bias=M1f[:, 1:2])

    # raw W tables [PAIRS, 1296] fp16 (adj applied after the collective)
    wblocks = [
        (0, [[36, 4], [1, 4]], "A", [[4, 4], [1, 4]]),
        (4, [[36, 4], [1, 32]], "B", [[32, 4], [1, 32]]),
        (144, [[1, 1152]], "C", [[1, 1152]]),
    ]
    for W, op_id, gR in ((WB, 0, grR), (WA, 1, glR)):
        off = lf_block_offsets(op_id)
        for (oo, od, key, idm) in wblocks:
            nc.vector.tensor_tensor(
                mk(W, PAIRS, oo, od),
                mk(beta1E, PAIRS, off[key], idm),
                mk(gR, PAIRS, oo, od),
                op=ALU.mult)

    # =======================================================================
    # Phase 3b (after the collective): lse, adjE2 = 2^(adj*log2e + K),
    # replicate, apply to W tables + lexical chart block 1.
    # =======================================================================
    nc.gpsimd.dma_start(sumexp_g[:], cc_out[:])
    nc.scalar.activation(lse21[:], sumexp_g[:], ACTF.Ln)
    # rearrange [128, NT] -> [1, CP]  (c = ct*128 + p) via DRAM bounce
    lse_d = dram.tile([128, NT], F32)
    nc.sync.dma_start(lse_d[:], lse21[:])
    nc.sync.dma_start(
        AP(lse_row.tensor, lse_row.offset,
           [[lse_row.ap[0][0], 1], [128, NT], [1, 128]]),
        AP(lse_d.tensor, lse_d.offset, [[lse_d.ap[0][0], 1], [1, NT], [NT, 128]]))
    nc.vector.tensor_tensor(adj[:], spN[:], lse_row[:], op=ALU.subtract)
    nc.vector.tensor_scalar(adj[:], adj[:], LOG2E, float(ADJK),
                            op0=ALU.mult, op1=ALU.add)
    adjE2 = p1.tile([1, CP], F16)
    nc.scalar.activation(adjE2[:], adj[:], ACTF.Exp, scale=LN2)
    adj_d = dram.tile([1, CP], F16)
    adjR = p1.tile([128, CP], F16)
    nc.sync.dma_start(adj_d[:], adjE2[:])
    nc.sync.dma_start(adjR[:],
                      AP(adj_d.tensor, adj_d.offset, [[0, 128], [1, CP]]))
    for W, op_id in ((WB, 0), (WA, 1)):
        off = lf_block_offsets(op_id)
        for (oo, od, key, idm) in wblocks:
            nc.vector.tensor_tensor(
                mk(W, PAIRS, oo, od),
                mk(W, PAIRS, oo, od),
                mk(adjR, PAIRS, off[key], idm),
                op=ALU.mult)

    # chart block 1 = beta1E' * adjE2; scale col 1 = M1i - K
    nc.vector.tensor_tensor(mk(chartA, PAIRS, BLK, [[1, 36]]),
                            beta1E[:, 0:NF], adjR[:, 0:NF], op=ALU.mult)
    nc.gpsimd.tensor_tensor(
        mk(chartA, PAIRS, BLK + 40, [[16, 2], [4, 4], [1, 4]]),
        mk(chartA, PAIRS, BLK + 4, [[16, 2], [4, 4], [1, 4]]),
        mk(gfb, PAIRS, 0, [[16, 2], [4, 4], [1, 4]]),
        op=ALU.mult)
    nc.vector.tensor_scalar_add(scaleA[:, 1:2], M1f[:, 0:1], float(-ADJK))
    # end-aligned block 1 is row-identical (end j = i+1 -> row (j-1)*4+b)
    nc.vector.tensor_copy(mk(chartE, PAIRS, (n - 1) * BLK, [[1, BLK]]),
                          mk(chartA, PAIRS, BLK, [[1, BLK]]))
    nc.vector.tensor_copy(scaleEr[:, n - 1:n], scaleA[:, 1:2])

    ph1.close()  # free ph1 tensors before the CKY working set

    es2 = contextlib.ExitStack()
    stage_pool = es2.enter_context(tc.tile_pool(name="stage", bufs=2))
    stageS_pool = es2.enter_context(tc.tile_pool(name="stageS", bufs=2))
    wash_pool = es2.enter_context(tc.tile_pool(name="wash", bufs=2))
    scr = es2.enter_context(tc.tile_pool(name="cky", bufs=2))
    scr1 = es2.enter_context(tc.tile_pool(name="cky1", bufs=1))

    # =======================================================================
    # Phase 4: CKY in scaled-exp space, fp16 data, integer pow2 scales.
    # chartA block L cols [L*BLK .. +72]: [0:36 chart | 40:56 FB | 56:72 FA]
    # chartE: same block layout at col (n-m)*BLK, rows indexed by span end.
    # scaleA col L / scaleEr col (n-m): integer scales as fp32.
    # true = stored * 2^scale;  G tables carry 2^GBOOST.
    # =======================================================================
    for L in range(2, n + 1):
        S = n - L + 1
        PS = 4 * S
        PSf = 4 * (S + 1)   # rows of the fresh block L-1
        NI = L - 2

        stageH = stage_pool.tile([128, (n + 1) * BLK], F16, tag="st")
        stageS = stageS_pool.tile([128, n + 2], F32, tag="sts")
        wash = wash_pool.tile([128, 1296], F16, tag="wa")

        # -- critical: fresh block L-1 (rows i+1 -> i), + its scale col.
        # chart cols [0:36] land first (unblock prodB); FA cols follow.
        nc.sync.dma_start(
            mk(stageS, PS, n - L + 1, [[1, 1]]),
            mk(scaleA, PS, L - 1, [[1, 1]], base_part=4))
        nc.sync.dma_start(
            mk(stageH, PS, (n - L + 1) * BLK, [[1, 36]]),
            mk(chartA, PS, (L - 1) * BLK, [[1, 36]], base_part=4))
        if NI > 0:
            nc.sync.dma_start(
                mk(stageH, PS, (n - L + 1) * BLK + 56, [[1, 16]]),
                mk(chartA, PS, (L - 1) * BLK + 56, [[1, 16]], base_part=4))
        # -- writeback fresh block L-1 to end-aligned history
        if L + 1 <= n:
            nc.sync.dma_start(
                mk(chartE, PSf, (n - L + 1) * BLK, [[1, BLK]],
                   base_part=4 * (L - 2)),
                mk(chartA, PSf, (L - 1) * BLK, [[1, BLK]]))
            nc.sync.dma_start(
                mk(scaleEr, PSf, n - L + 1, [[1, 1]], base_part=4 * (L - 2)),
                mk(scaleA, PSf, L - 1, [[1, 1]]))
        # -- prefetchable history: blocks 1..L-2 (+ scales)
        if L >= 3:
            nc.sync.dma_start(
                mk(stageH, PS, (n - L + 2) * BLK, [[1, (L - 2) * BLK]]),
                mk(chartE, PS, (n - L + 2) * BLK, [[1, (L - 2) * BLK]],
                   base_part=4 * (L - 1)))
            nc.sync.dma_start(
                mk(stageS, PS, n - L + 2, [[1, L - 2]]),
                mk(scaleEr, PS, n - L + 2, [[1, L - 2]],
                   base_part=4 * (L - 1)))
        # -- wash: WA rows at the right-end pair
        nc.sync.dma_start(
            mk(wash, PS, 0, [[1, 1296]]),
            mk(WA, PS, 0, [[1, 1296]], base_part=4 * (L - 1)))

        # ---- edge-A products (free of the critical DMAs)
        prodAB = scr1.tile([128, 2592], F16, tag="prod")
        nc.vector.tensor_tensor(
            prodAB[0:PS, 0:1296],
            mk(wash, PS, 0, [[1, 1296]]),
            mk(chartA, PS, (L - 1) * BLK, [[0, 36], [1, 36]]),
            op=ALU.mult)

        # ---- edge-B products (right after the fresh chart DMA lands)
        nc.vector.tensor_tensor(
            prodAB[0:PS, 1296:2592],
            mk(WB, PS, 0, [[1, 1296]]),
            mk(stageH, PS, (n - L + 1) * BLK, [[0, 36], [1, 36]]),
            op=ALU.mult)
        red72 = scr.tile([128, 160], F16, tag="red")
        nc.vector.tensor_reduce(red72[0:PS, 0:36],
                                mk(prodAB, PS, 0, [[36, 36], [1, 36]]),
                                axis=AXIS.X, op=ALU.add)
        nc.vector.tensor_reduce(red72[0:PS, 36:72],
                                mk(prodAB, PS, 1296, [[36, 36], [1, 36]]),
                                axis=AXIS.X, op=ALU.add)

        # ---- scales: sAsm[k] = sI_k, k=1..L-1; note the edge cases reuse
        # interior scales: sA = sI_{L-1} (k=L-1 split), sB = sI_1.
        sAsm = scr.tile([128, n + 8], F32, tag="sasm")
        eBits = scr.tile([128, n + 8], I32, tag="ebits")
        mstarN = scr.tile([128, 2], F32, tag="mstar")
        nc.vector.tensor_tensor(
            sAsm[0:PS, 1:L],
            scaleA[0:PS, 1:L],
            stageS[0:PS, n - L + 1:n], op=ALU.add)
        nc.vector.tensor_reduce(mstarN[0:PS, 0:1], sAsm[0:PS, 1:L],
                                axis=AXIS.X, op=ALU.max, negate=True)
        nc.vector.tensor_scalar(sAsm[0:PS, 1:L], sAsm[0:PS, 1:L],
                                mstarN[0:PS, 0:1], -126.0,
                                op0=ALU.add, op1=ALU.max)
        nc.vector.tensor_scalar(eBits[0:PS, 1:L], sAsm[0:PS, 1:L],
                                127.0, 8388608.0, op0=ALU.add, op1=ALU.mult)
        eAll = lambda off, dims: mk(eBits, PS, off, dims).bitcast(F32)

        # ---- interior terms (res<4), batched over k, on GpSimd
        if NI > 0:
            tI = scr1.tile([128, 2 * max(n - 2, 1) * 16], F16, tag="ti")
            nc.gpsimd.tensor_tensor(   # IA: chart[k][i] args x stage FA(L-k)
                mk(tI, PS, 0, [[2 * NI * 4, 4], [4, NI], [1, 4]]),
                mk(chartA, PS, BLK, [[0, 4], [BLK, NI], [1, 4]]),
                mk(stageH, PS, (n - L + 1) * BLK + 56,
                   [[4, 4], [BLK, NI], [1, 4]]),
                op=ALU.mult)
            nc.gpsimd.tensor_tensor(   # IB: stage args(L-k) x chart[k] FB
                mk(tI, PS, NI * 4, [[2 * NI * 4, 4], [4, NI], [1, 4]]),
                mk(stageH, PS, (n - L + 2) * BLK, [[0, 4], [BLK, NI], [1, 4]]),
                mk(chartA, PS, 2 * BLK + 40, [[4, 4], [BLK, NI], [1, 4]]),
                op=ALU.mult)
            for half in range(2):      # x eI (k scales), in place
                nc.gpsimd.tensor_tensor(
                    mk(tI, PS, half * NI * 4,
                       [[2 * NI * 4, 4], [4, NI], [1, 4]]),
                    mk(tI, PS, half * NI * 4,
                       [[2 * NI * 4, 4], [4, NI], [1, 4]]),
                    eAll(1 + half, [[0, 4], [1, NI], [0, 4]]),
                    op=ALU.mult)
            nc.gpsimd.tensor_tensor(   # fold IB half into IA half
                mk(tI, PS, 0, [[2 * NI * 4, 4], [4, NI], [1, 4]]),
                mk(tI, PS, 0, [[2 * NI * 4, 4], [4, NI], [1, 4]]),
                mk(tI, PS, NI * 4, [[2 * NI * 4, 4], [4, NI], [1, 4]]),
                op=ALU.add)

        # ---- combine cases with their pow2 scale factors (eA at col L-1,
        # eB at col 1: negative-stride pair view)
        nc.vector.tensor_tensor(red72[0:PS, 72:144], red72[0:PS, 0:72],
                                eAll(L - 1, [[-(L - 2), 2], [0, 36]]),
                                op=ALU.mult)
        tot36 = scr.tile([128, 40], F16, tag="tot")
        nc.vector.tensor_tensor(tot36[0:PS, 0:36], red72[0:PS, 72:108],
                                red72[0:PS, 108:144], op=ALU.add)
        if NI > 0:
            nc.vector.tensor_reduce(   # sum over (k, arg) -> [PS, 4]
                tot36[0:PS, 36:40],
                mk(tI, PS, 0,
                   [[2 * NI * 4, 4], [4, NI], [1, 4]]),
                axis=AXIS.XY, op=ALU.add)
            nc.vector.tensor_tensor(tot36[0:PS, 0:4], tot36[0:PS, 0:4],
                                    tot36[0:PS, 36:40], op=ALU.add)

        # ---- pow2 renorm and write chart block L
        # scale carries a +(127+GBOOST) per-level offset, removed at the end
        mvE = scr.tile([128, 4], F32, tag="mval")
        mvI = lambda c: mvE[0:PS, c:c + 1].bitcast(I32)
        nc.vector.tensor_reduce(mvE[0:PS, 0:1], tot36[0:PS, 0:36],
                                axis=AXIS.X, op=ALU.max)
        nc.vector.tensor_scalar(mvI(2), mvI(0), expMask[0:PS, 0:1], None,
                                op0=ALU.bitwise_and)
        nc.vector.tensor_scalar(scaleA[0:PS, L:L + 1], mvI(2),
                                1.0 / 8388608.0, mstarN[0:PS, 0:1],
                                op0=ALU.mult, op1=ALU.subtract)
        nc.vector.reciprocal(mvE[0:PS, 1:2], mvE[0:PS, 2:3])
        nc.vector.tensor_scalar(
            chartA[0:PS, L * BLK:L * BLK + 36],
            tot36[0:PS, 0:36], mvE[0:PS, 1:2], None, op0=ALU.mult)
        nc.gpsimd.tensor_tensor(
            mk(chartA, PS, L * BLK + 40, [[16, 2], [4, 4], [1, 4]]),
            mk(chartA, PS, L * BLK + 4, [[16, 2], [4, 4], [1, 4]]),
            mk(gfb, PS, 0, [[16, 2], [4, 4], [1, 4]]),
            op=ALU.mult)

    # =======================================================================
    # Phase 5: root -> nll per sentence
    # =======================================================================
    nc.vector.tensor_tensor(finh[:, 0:4],
                            mk(chartA, 4, n * BLK, [[1, 4]]),
                            rsRep[:], op=ALU.mult)
    nc.vector.tensor_reduce(fin[:, 4:5], finh[:, 0:4], axis=AXIS.X,
                            op=ALU.add)
    nc.scalar.activation(fin[:, 5:6], fin[:, 4:5], ACTF.Ln)
    # nll = -(ln(fin) + scale * ln2); scale carries +(127+GBOOST)(n-1)
    nc.vector.scalar_tensor_tensor(
        fin[:, 6:7], scaleA[0:4, n:n + 1], -LN2,
        fin[:, 5:6], op0=ALU.mult, op1=ALU.subtract)
    nc.vector.tensor_scalar_add(fin[:, 7:8], fin[:, 6:7],
                                float((127 + GBOOST) * (n - 1)) * LN2)
    nc.sync.dma_start(d["out"][:], fin[:, 7:8])
    es2.close()
    es.close()


# ============================================================== host wrapper
_PROG_CACHE = {}


def _get_program(cfg: Cfg):
    key = (cfg.n, cfg.v_loc, cfg.n_cores)
    if key not in _PROG_CACHE:
        _PROG_CACHE[key] = build_program(cfg)
    return _PROG_CACHE[key]


def make_inmaps(cfg: Cfg, inputs):
    """Host-side shard/pack of FULL inputs -> per-core DRAM input dicts."""
    import ml_dtypes
    bf16 = ml_dtypes.bfloat16
    x = np.asarray(inputs["x"])
    check_functor_tables(np.asarray(inputs["l_functors"]),
                         np.asarray(inputs["r_functors"]))
    nt_emb = np.asarray(inputs["nt_emb"], np.float32)          # [C, D]
    vocab_W = np.asarray(inputs["vocab_W"], np.float32)        # [D, V]
    vocab_b = np.asarray(inputs["vocab_b"], np.float32)        # [V]

    ntembT = np.zeros((65, CP), bf16)
    ntembT[0:64, 0:C] = nt_emb.T.astype(bf16)
    ntembT[64, :] = 1.0

    mlpW = np.zeros((64, 322), bf16)
    for j, k in enumerate(("sW1", "r1W1", "r1W2", "r2W1", "r2W2")):
        mlpW[:, j * 64:(j + 1) * 64] = np.asarray(inputs[k], np.float32)
    mlpW[:, 320:322] = np.asarray(inputs["sW2"], np.float32)

    mlpB = np.zeros((64, 8), np.float32)
    for j, k in enumerate(("sb1", "r1b1", "r1b2", "r2b1", "r2b2")):
        mlpB[:, j] = np.asarray(inputs[k], np.float32)

    ruleWb = np.zeros((36, 144), np.float32)
    ruleWb[:, 0:72] = np.asarray(inputs["rule_W"], np.float32)
    ruleWb[:, 72:144] = np.tile(
        np.asarray(inputs["rule_b"], np.float32)[None, :], (36, 1))

    smallv = np.zeros((1, 16), np.float32)
    smallv[0, 0:2] = np.asarray(inputs["sb2"], np.float32)
    smallv[0, 2:6] = np.asarray(inputs["root_W"], np.float32)[0, 0:4]
    smallv[0, 6:10] = np.asarray(inputs["root_b"], np.float32)[0:4]

    vs = cfg.v_loc
    in_maps = []
    for core in range(cfg.n_cores):
        vocabW = np.zeros((65, cfg.v_pad), bf16)
        vocabW[64, :] = NEGB
        vocabW[0:64, 0:vs] = vocab_W[:, core * vs:(core + 1) * vs]
        vocabW[64, 0:vs] = vocab_b[core * vs:(core + 1) * vs]

        words = x[core * BLOC:(core + 1) * BLOC, 0:cfg.n]   # [BLOC, n]
        wid = words.T.reshape(-1)                           # pair = i*4 + b
        wordW = np.zeros((66, cfg.pairs), bf16)
        wordW[0:64, :] = vocab_W[:, wid].astype(bf16)
        wordW[64, :] = vocab_b[wid]
        wordW[65, :] = 1.0

        in_maps.append({
            "ntembT": ntembT, "vocabW": vocabW, "wordW": wordW,
            "mlpW": mlpW, "mlpB": mlpB, "ruleWb": ruleWb, "smallv": smallv,
        })
    return in_maps


def kernel(**inputs) -> np.ndarray:
    cfg = Cfg(n=32, v_loc=V // NCORES, n_cores=NCORES)
    nc = _get_program(cfg)
    in_maps = make_inmaps(cfg, inputs)
    res = bass_utils.run_bass_kernel_spmd(
        nc, in_maps, core_ids=list(range(cfg.n_cores)))
    out = np.concatenate([r["out_nll"].reshape(-1) for r in res.results])
    return out.astype(np.float32)


if __name__ == "__main__":
    from reference import setup_inputs, reference
    inputs = {k: np.asarray(v) for k, v in setup_inputs().items()}
    got = kernel(**inputs)
    exp = np.asarray(reference(**inputs))
    rel = np.max(np.abs(got - exp) / np.maximum(np.abs(exp), 1e-6))
    print("expected:", exp[:8])
    print("got     :", got[:8])
    print("Relative error:", rel)


# revision 67
# speedup vs baseline: 1.4398x; 1.0131x over previous
"""Trainium2 Bass kernel for nn_BasicCGInducer (CKY inside algorithm for a
categorial-grammar inducer).

Strategy (8 NeuronCores):
  - Data-parallel over sentences: core j handles sentences 4j..4j+3.
  - Emission log-partition ([C,V] softmax denominator) is tensor-parallel
    over vocab: each core exps a 4000-column V-shard, one AllReduce of [C].
  - CKY inside pass runs per-core in scaled-exp space with POWER-OF-TWO
    integer span scales: the per-level rescales are pure DVE bit/int ops
    (no activation-table Exp/Ln in the loop), and all chart data, rule
    tables and products are fp16 to engage the DVE 2x/4x perf modes.
  - All matmuls (emission, split-MLP, beta1) run in bf16.

kernel(**inputs) takes FULL inputs, shards on host, runs one SPMD bass
program on cores 0-7, and reassembles the [32] output.
"""
import sys
import contextlib

sys.path.insert(0, "/opt/trn_rl_repo")

import numpy as np

import concourse.bass as bass
import concourse.bacc as bacc
import concourse.mybir as mybir
import concourse.tile as tile
from concourse.ap import AP
from concourse import bass_utils

F32 = mybir.dt.float32
F16 = mybir.dt.float16
BF16 = mybir.dt.bfloat16
I32 = mybir.dt.int32
ALU = mybir.AluOpType
ACTF = mybir.ActivationFunctionType
AXIS = mybir.AxisListType

# ---------------------------------------------------------------- constants
P4 = 4          # primitive cats
NF = 36         # non-functor cats
C = 2596        # total cats
CP = 2688       # padded C (21 * 128)
NT = CP // 128  # 21 c-tiles
D = 64
B = 32          # total sentences
NCORES = 8
BLOC = B // NCORES  # 4 sentences per core
V = 32000
BLK = 72        # chart block: [0:36 chart | 36:40 pad | 40:56 FB | 56:72 FA]
NEGB = -30.0    # bias for padded vocab columns (exp(-30) ~ 0 vs Z ~ 6e3)
GBOOST = 10     # G tables carry 2^GBOOST; span scale compensates
LN2 = 0.6931471805599453
LOG2E = 1.4426950408889634
# Schraudolph fast-exp: bitcast_f32(int32(x * 2^23/ln2 + b)), mean-unbiased b
SCH_A = 12102203.161561485
SCH_B = 1064866805.0
ADJK = 16       # 2^ADJK boost on the emission adj factor (lse ~ 10.9 nats)


class Cfg:
    def __init__(self, n=32, v_loc=4000, n_cores=8):
        self.n = n                      # sentence length
        self.v_loc = v_loc              # vocab shard per core
        self.v_pad = ((v_loc + 511) // 512) * 512
        self.n_cores = n_cores
        self.pairs = 4 * n              # (i, b) pairs on partitions


# ------------------------------------------------------------ functor maps
def lf_block_offsets(op):
    """c = off + {A: 4r+a | B: 32r+(a-4) | C: 36(r-4)+a} per derivation of
    the deterministic functor-id tables. op=0 -> l_functors, 1 -> r_functors."""
    return {
        "A": 4 + 16 * op,            # res<4, arg<4 : c = A + 4*res + arg
        "B": 36 + 1280 * op,         # res<4, arg>=4: c = B + 32*res + (arg-4)
        "C": 164 + 1280 * op,        # res>=4      : c = C0 + 36*(res-4) + arg
    }


def check_functor_tables(l_functors, r_functors):
    for op, tab in ((0, l_functors), (1, r_functors)):
        off = lf_block_offsets(op)
        exp = np.zeros((NF, NF), np.int64)  # [arg, res]
        for res in range(NF):
            for arg in range(NF):
                if res < P4 and arg < P4:
                    exp[arg, res] = off["A"] + 4 * res + arg
                elif res < P4:
                    exp[arg, res] = off["B"] + 32 * res + (arg - 4)
                else:
                    exp[arg, res] = off["C"] + 36 * (res - 4) + arg
        assert np.array_equal(np.asarray(tab, np.int64), exp), (
            f"functor table structure mismatch (op={op})")


# ---------------------------------------------------------------- AP helper
def mk(t, parts, off, dims, base_part=0):
    """Raw AP on tile t: partition range [base_part, base_part+parts),
    free offset `off` (elements), extra free dims [[step, count], ...]."""
    w = t.ap[0][0]
    return AP(t.tensor, t.offset + base_part * w + off, [[w, parts]] + dims)


# ============================================================ device program
def build_program(cfg: Cfg):
    nc = bacc.Bacc("TRN2", target_bir_lowering=False, debug=False,
                   num_devices=cfg.n_cores)
    d = {
        "ntembT": nc.dram_tensor("ntembT", [65, CP], BF16,
                                 kind="ExternalInput"),
        "vocabW": nc.dram_tensor("vocabW", [65, cfg.v_pad], BF16,
                                 kind="ExternalInput"),
        "wordW": nc.dram_tensor("wordW", [66, cfg.pairs], BF16,
                                kind="ExternalInput"),
        "mlpW": nc.dram_tensor("mlpW", [64, 322], BF16, kind="ExternalInput"),
        "mlpB": nc.dram_tensor("mlpB", [64, 8], F32, kind="ExternalInput"),
        "ruleWb": nc.dram_tensor("ruleWb", [36, 144], F32,
                                 kind="ExternalInput"),
        "smallv": nc.dram_tensor("smallv", [1, 16], F32,
                                 kind="ExternalInput"),
        "out": nc.dram_tensor("out_nll", [BLOC, 1], F32,
                              kind="ExternalOutput"),
    }
    with tile.TileContext(nc) as tc:
        with nc.allow_low_precision(reason="fp16 CKY inside pass"):
            _trace(tc, cfg, d)
    nc.compile()
    return nc


def _trace(tc, cfg, d):
    nc = tc.nc
    n, PAIRS, VP = cfg.n, cfg.pairs, cfg.v_pad
    NV = VP // 512                    # 512-col v-tiles per core
    NHALF = (NV + 3) // 4             # ACT chunks of up to 4 v-tiles
    HW = CP // 2                      # MLP half width (1344)

    es = contextlib.ExitStack()
    keep = es.enter_context(tc.tile_pool(name="keep", bufs=1))
    dram = es.enter_context(tc.tile_pool(name="dram", bufs=1, space="DRAM"))

    # ---------------- long-lived tensors
    chartA = keep.tile([PAIRS, (n + 1) * BLK], F16)
    chartE = keep.tile([PAIRS, (n + 1) * BLK], F16)
    scaleA = keep.tile([PAIRS, n + 2], F32)   # col L = scale of chart[L]
    scaleEr = keep.tile([PAIRS, n + 2], F32)  # col n-m = scale of end block m
    WA = keep.tile([PAIRS, 1296], F16)
    WB = keep.tile([PAIRS, 1296], F16)
    glR = keep.tile([128, 1296], F16)
    grR = keep.tile([128, 1296], F16)
    gfb = keep.tile([128, 32], F16)           # [0:16 grR 4x4 | 16:32 glR 4x4]
    M1f = keep.tile([PAIRS, 4], F32)          # 0: M1i (int-val), 1: -ln2*M1i
    m1i = keep.tile([PAIRS, 2], I32)
    mlpB = keep.tile([64, 8], F32)
    smallv = keep.tile([1, 16], F32)
    sumexp_parts = keep.tile([128, NT * NHALF], F32)
    sumexp_loc = keep.tile([128, NT], F32)
    sumexp_g = keep.tile([128, NT], F32)
    lse21 = keep.tile([128, NT], F32)
    s0E = keep.tile([1, NF], F32)
    db = keep.tile([1, 2], F32)
    rsRep = keep.tile([4, 4], F16)
    fin = keep.tile([4, 8], F32)
    finh = keep.tile([4, 4], F16)
    expMask = keep.tile([128, 1], I32)
    nc.gpsimd.memset(expMask[:], 0x7F800000)

    nc.sync.dma_start(mlpB[:], d["mlpB"][:])
    nc.sync.dma_start(smallv[:], d["smallv"][:])
    nc.gpsimd.memset(chartA[:], 0.0)
    nc.gpsimd.memset(chartE[:], 0.0)
    nc.gpsimd.memset(scaleA[:], 0.0)
    nc.gpsimd.memset(scaleEr[:], 0.0)

    ph1 = contextlib.ExitStack()
    p1 = ph1.enter_context(tc.tile_pool(name="ph1", bufs=1))
    ntembT = p1.tile([65, CP], BF16)
    vocabW = p1.tile([65, VP], BF16)
    wordW = p1.tile([66, PAIRS], BF16)
    mlpW = p1.tile([64, 322], BF16)
    ruleWb = p1.tile([36, 144], F32)
    lse_row = p1.tile([1, CP], F32)
    adj = p1.tile([1, CP], F32)
    spN = p1.tile([1, CP], F32)
    beta1E = p1.tile([PAIRS, CP], F16)
    ruleflat = p1.tile([1, 36 * 72], F32)

    nc.sync.dma_start(ntembT[:], d["ntembT"][:])
    nc.sync.dma_start(vocabW[:], d["vocabW"][:])
    nc.sync.dma_start(wordW[:], d["wordW"][:])
    nc.sync.dma_start(mlpW[:], d["mlpW"][:])
    nc.sync.dma_start(ruleWb[:], d["ruleWb"][:])

    # =======================================================================
    # Phase 1: emission partition function. exp chunks split between the
    # scalar engine (table exp + accum) and DVE (Schraudolph bitcast exp).
    # =======================================================================
    with tc.tile_pool(name="psum_e", bufs=2, space="PSUM") as pse, \
         tc.tile_pool(name="scr_e", bufs=2) as scre:
        for ct in range(NT):
            for h in range(NHALF):
                vt0 = h * 4
                nvt = min(4, NV - vt0)
                pt = pse.tile([128, 512 * nvt], F32, tag="pse")
                for vt in range(nvt):
                    nc.tensor.matmul(
                        pt[:, vt * 512:(vt + 1) * 512],
                        ntembT[:, ct * 128:(ct + 1) * 128],
                        vocabW[:, (vt0 + vt) * 512:(vt0 + vt + 1) * 512],
                        start=True, stop=True)
                acc = sumexp_parts[:, ct * NHALF + h:ct * NHALF + h + 1]
                ci = ct * NHALF + h
                if (ci * 15) % 42 >= 15:  # 27: scalar-engine exp + accum
                    sce = scre.tile([128, 512 * 4], F32, tag="scre")
                    nc.scalar.activation(
                        sce[:, 0:512 * nvt], pt[:], ACTF.Exp, accum_out=acc)
                else:                 # 14: DVE Schraudolph exp + sum
                    sci = scre.tile([128, 512 * 4], I32, tag="scri")
                    nc.vector.tensor_scalar(
                        sci[:, 0:512 * nvt], pt[:], SCH_A, SCH_B,
                        op0=ALU.mult, op1=ALU.add)
                    nc.vector.tensor_reduce(
                        acc, sci[:, 0:512 * nvt].bitcast(F32),
                        axis=AXIS.X, op=ALU.add)

    if NHALF > 1:
        nc.vector.tensor_reduce(
            sumexp_loc[:],
            mk(sumexp_parts, 128, 0, [[NHALF, NT], [1, NHALF]]),
            axis=AXIS.X, op=ALU.add)
    else:
        nc.vector.tensor_copy(sumexp_loc[:], sumexp_parts[:, 0:NT])

    # AllReduce over cores via DRAM bounce
    cc_in = dram.tile([128, NT], F32)
    cc_out = dram.tile([128, NT], F32)
    nc.sync.dma_start(cc_in[:], sumexp_loc[:])
    nc.gpsimd.collective_compute(
        "AllReduce", ALU.add,
        replica_groups=[list(range(cfg.n_cores))],
        ins=[cc_in[:].opt()], outs=[cc_out[:].opt()])
    # the collective-dependent lse/adj tail is issued after phase 3's
    # collective-independent work so no engine queue blocks on it.

    # =======================================================================
    # Phase 2: split MLP (transposed layout hT [64, *]), rule tables, root
    # =======================================================================
    nc.vector.tensor_tensor(db[:, 0:1], smallv[:, 0:1], smallv[:, 1:2],
                            op=ALU.subtract)
    nc.vector.tensor_scalar_mul(db[:, 1:2], db[:, 0:1], -1.0)

    with tc.tile_pool(name="mlp", bufs=1) as mlp:
        hA = mlp.tile([64, HW], BF16, tag="hA")
        hB = mlp.tile([64, HW], BF16, tag="hB")
        hC = mlp.tile([64, HW], BF16, tag="hC")
        s_rows = mlp.tile([2, HW], F32, tag="srows")
        w1 = mlp.tile([1, HW], F32, tag="w1")
        w2 = mlp.tile([1, HW], F32, tag="w2")
        w3 = mlp.tile([1, HW], F32, tag="w3")

        for half in range(2):
            base = half * HW

            def dense_relu(dst, col0, rhs, bias_col, res_add=None, rb=0,
                           relu=True):
                with tc.tile_pool(name="psum_m", bufs=2,
                                  space="PSUM") as psm:
                    for c0 in range(0, HW, 512):
                        c1 = min(c0 + 512, HW)
                        pm = psm.tile([64, 512], F32, tag="psm")
                        nc.tensor.matmul(pm[:, 0:c1 - c0],
                                         mlpW[:, col0:col0 + 64],
                                         rhs[0:64, rb + c0:rb + c1],
                                         start=True, stop=True)
                        nc.scalar.activation(
                            dst[:, c0:c1], pm[:, 0:c1 - c0],
                            ACTF.Relu if relu else ACTF.Identity,
                            bias=mlpB[0:64, bias_col:bias_col + 1])
                        if res_add is not None:
                            nc.vector.tensor_tensor(
                                dst[:, c0:c1], dst[:, c0:c1],
                                res_add[:, c0:c1], op=ALU.add)

            dense_relu(hA, 0, ntembT, 0, rb=base,
                       relu=False)                      # h1 (linear)
            dense_relu(hB, 64, hA, 1)                   # t = relu(h1 W + b)
            dense_relu(hC, 128, hB, 2, res_add=hA)      # h2
            dense_relu(hB, 192, hC, 3)                  # t2
            dense_relu(hA, 256, hB, 4, res_add=hC)      # h3

            with tc.tile_pool(name="psum_s", bufs=2, space="PSUM") as pss:
                for c0 in range(0, HW, 512):
                    c1 = min(c0 + 512, HW)
                    ps = pss.tile([2, 512], F32, tag="pss")
                    nc.tensor.matmul(ps[:, 0:c1 - c0], mlpW[:, 320:322],
                                     hA[0:64, c0:c1], start=True, stop=True)
                    nc.vector.tensor_copy(s_rows[:, c0:c1], ps[:, 0:c1 - c0])

            # d = s0 - s1 (s1 via DMA to partition 0)
            nc.sync.dma_start(w1[:], s_rows[1:2, :])
            nc.vector.tensor_tensor(w2[:], s_rows[0:1, :], w1[:],
                                    op=ALU.subtract)
            # y = d + db;  softplus(y) = max(y,0) + ln(1+exp(-|y|))
            y = w2
            nc.vector.tensor_scalar_add(y[:], y[:], db[:, 0:1])
            nc.scalar.activation(w1[:], y[:], ACTF.Abs)
            nc.scalar.activation(w1[:], w1[:], ACTF.Exp, scale=-1.0)
            nc.scalar.activation(w1[:], w1[:], ACTF.Ln, bias=1.0)
            nc.vector.tensor_scalar_max(w3[:], y[:], 0.0)
            nc.vector.tensor_tensor(w3[:], w3[:], w1[:], op=ALU.add)  # sp
            # spN = -softplus(y); adj = spN - lse comes after the collective
            nc.vector.tensor_scalar_mul(spN[:, base:base + HW], w3[:], -1.0)
            if half == 0:
                # split0 = -softplus(-y) = y - softplus(y); split0E = exp
                nc.vector.tensor_tensor(s0E[:], y[:, 0:NF], w3[:, 0:NF],
                                        op=ALU.subtract)
                nc.scalar.activation(s0E[:], s0E[:], ACTF.Exp)

    # rule tables: softmax over 72 per res row
    rsum = keep.tile([36, 72], F32)
    rmax = keep.tile([36, 2], F32)
    rsumexp = keep.tile([36, 2], F32)
    nc.vector.tensor_tensor(rsum[:], ruleWb[:, 0:72], ruleWb[:, 72:144],
                            op=ALU.add)
    nc.vector.tensor_reduce(rmax[:, 0:1], rsum[:], axis=AXIS.X, op=ALU.max)
    nc.vector.tensor_scalar_mul(rmax[:, 1:2], rmax[:, 0:1], -1.0)
    nc.scalar.activation(rsum[:], rsum[:], ACTF.Exp, bias=rmax[:, 1:2],
                         accum_out=rsumexp[:, 0:1])
    nc.vector.reciprocal(rsumexp[:, 1:2], rsumexp[:, 0:1])
    nc.vector.tensor_scalar_mul(rsum[:], rsum[:], rsumexp[:, 1:2])

    # flatten ruleEn to [1, 2592] via DRAM, then fp16 G-flats (x 2^GBOOST)
    rule_d = dram.tile([36, 72], F32)
    nc.sync.dma_start(rule_d[:], rsum[:])
    nc.sync.dma_start(
        AP(ruleflat.tensor, ruleflat.offset,
           [[ruleflat.ap[0][0], 1], [1, 36 * 72]]),
        rule_d[:])
    g_d = dram.tile([2, 1296], F16)
    gtmp = keep.tile([1, 1296], F16)
    for row, off in ((0, 0), (1, 36)):   # 0: Gl (larg), 1: Gr (rarg)
        nc.vector.scalar_tensor_tensor(
            gtmp[:],
            mk(ruleflat, 1, off, [[72, 36], [1, 36]]),
            float(1 << GBOOST),
            mk(s0E, 1, 0, [[1, 36], [0, 36]]),
            op0=ALU.mult, op1=ALU.mult)
        nc.sync.dma_start(g_d[row:row + 1, :], gtmp[:])
    for dstt, row in ((glR, 0), (grR, 1)):
        nc.sync.dma_start(
            dstt[:],
            AP(g_d.tensor, g_d.offset + row * g_d.ap[0][0],
               [[0, 128], [1, 1296]]))
    nc.vector.tensor_copy(gfb[:, 0:16], mk(grR, 128, 0, [[36, 4], [1, 4]]))
    nc.vector.tensor_copy(gfb[:, 16:32], mk(glR, 128, 0, [[36, 4], [1, 4]]))

    # root: rsEn = softmax(root_W[0,0:4] + root_b[0:4]) replicated to 4 parts
    rs4 = keep.tile([1, 8], F32)
    rsE = keep.tile([1, 8], F32)
    rsEh = keep.tile([1, 4], F16)
    nc.vector.tensor_tensor(rs4[:, 0:4], smallv[:, 2:6], smallv[:, 6:10],
                            op=ALU.add)
    nc.vector.tensor_reduce(rs4[:, 4:5], rs4[:, 0:4], axis=AXIS.X, op=ALU.max)
    nc.vector.tensor_scalar_mul(rs4[:, 5:6], rs4[:, 4:5], -1.0)
    nc.scalar.activation(rsE[:, 0:4], rs4[:, 0:4], ACTF.Exp,
                         bias=rs4[:, 5:6], accum_out=rsE[:, 4:5])
    nc.vector.reciprocal(rsE[:, 5:6], rsE[:, 4:5])
    nc.vector.tensor_scalar_mul(rsE[:, 0:4], rsE[:, 0:4], rsE[:, 5:6])
    nc.vector.tensor_copy(rsEh[:], rsE[:, 0:4])
    rs_d = dram.tile([1, 4], F16)
    nc.sync.dma_start(rs_d[:], rsEh[:])
    nc.sync.dma_start(rsRep[:],
                      AP(rs_d.tensor, rs_d.offset, [[0, 4], [1, 4]]))

    # =======================================================================
    # Phase 3a (collective-independent): pb0 = wordW.T @ ntembT (no adj),
    # pow2 M1, beta1E' = 2^(pb0*log2e - M1i), raw W tables.
    # =======================================================================
    with tc.tile_pool(name="psum_b", bufs=1, space="PSUM") as psb:
        pb = psb.tile([PAIRS, CP], F32)
        for c0 in range(0, CP, 512):
            c1 = min(c0 + 512, CP)
            nc.tensor.matmul(pb[:, c0:c1], wordW[0:65, :], ntembT[:, c0:c1],
                             start=True, stop=True)
        # M1i = int(max(pb) * log2e); beta1E = 2^(pb*log2e - M1i)
        nc.vector.tensor_reduce(M1f[:, 2:3], pb[:, 0:C], axis=AXIS.X,
                                op=ALU.max)
        nc.vector.tensor_scalar(m1i[:, 0:1], M1f[:, 2:3], LOG2E, None,
                                op0=ALU.mult)                  # f32 -> i32
        nc.vector.tensor_scalar(M1f[:, 0:1], m1i[:, 0:1], 1.0, None,
                                op0=ALU.mult)                  # i32 -> f32
        nc.vector.tensor_scalar_mul(M1f[:, 1:2], M1f[:, 0:1], -LN2)
        nc.scalar.activation(beta1E[:], pb[:], ACTF.Exp,
                             bias=M1f[:, 1:2])

    # raw W tables [PAIRS, 1296] fp16 (adj applied after the collective)
    wblocks = [
        (0, [[36, 4], [1, 4]], "A", [[4, 4], [1, 4]]),
        (4, [[36, 4], [1, 32]], "B", [[32, 4], [1, 32]]),
        (144, [[1, 1152]], "C", [[1, 1152]]),
    ]
    for W, op_id, gR in ((WB, 0, grR), (WA, 1, glR)):
        off = lf_block_offsets(op_id)
        for (oo, od, key, idm) in wblocks:
            nc.vector.tensor_tensor(
                mk(W, PAIRS, oo, od),
                mk(beta1E, PAIRS, off[key], idm),
                mk(gR, PAIRS, oo, od),
                op=ALU.mult)

    # =======================================================================
    # Phase 3b (after the collective): lse, adjE2 = 2^(adj*log2e + K),
    # replicate, apply to W tables + lexical chart block 1.
    # =======================================================================
    nc.gpsimd.dma_start(sumexp_g[:], cc_out[:])
    nc.scalar.activation(lse21[:], sumexp_g[:], ACTF.Ln)
    # rearrange [128, NT] -> [1, CP]  (c = ct*128 + p) via DRAM bounce
    lse_d = dram.tile([128, NT], F32)
    nc.sync.dma_start(lse_d[:], lse21[:])
    nc.sync.dma_start(
        AP(lse_row.tensor, lse_row.offset,
           [[lse_row.ap[0][0], 1], [128, NT], [1, 128]]),
        AP(lse_d.tensor, lse_d.offset, [[lse_d.ap[0][0], 1], [1, NT], [NT, 128]]))
    nc.vector.tensor_tensor(adj[:], spN[:], lse_row[:], op=ALU.subtract)
    nc.vector.tensor_scalar(adj[:], adj[:], LOG2E, float(ADJK),
                            op0=ALU.mult, op1=ALU.add)
    adjE2 = p1.tile([1, CP], F16)
    nc.scalar.activation(adjE2[:], adj[:], ACTF.Exp, scale=LN2)
    adj_d = dram.tile([1, CP], F16)
    adjR = p1.tile([128, CP], F16)
    nc.sync.dma_start(adj_d[:], adjE2[:])
    nc.sync.dma_start(adjR[:],
                      AP(adj_d.tensor, adj_d.offset, [[0, 128], [1, CP]]))
    for W, op_id in ((WB, 0), (WA, 1)):
        off = lf_block_offsets(op_id)
        for (oo, od, key, idm) in wblocks:
            nc.vector.tensor_tensor(
                mk(W, PAIRS, oo, od),
                mk(W, PAIRS, oo, od),
                mk(adjR, PAIRS, off[key], idm),
                op=ALU.mult)

    # chart block 1 = beta1E' * adjE2; scale col 1 = M1i - K
    nc.vector.tensor_tensor(mk(chartA, PAIRS, BLK, [[1, 36]]),
                            beta1E[:, 0:NF], adjR[:, 0:NF], op=ALU.mult)
    nc.gpsimd.tensor_tensor(
        mk(chartA, PAIRS, BLK + 40, [[16, 2], [4, 4], [1, 4]]),
        mk(chartA, PAIRS, BLK + 4, [[16, 2], [4, 4], [1, 4]]),
        mk(gfb, PAIRS, 0, [[16, 2], [4, 4], [1, 4]]),
        op=ALU.mult)
    nc.vector.tensor_scalar_add(scaleA[:, 1:2], M1f[:, 0:1], float(-ADJK))
    # end-aligned block 1 is row-identical (end j = i+1 -> row (j-1)*4+b)
    nc.vector.tensor_copy(mk(chartE, PAIRS, (n - 1) * BLK, [[1, BLK]]),
                          mk(chartA, PAIRS, BLK, [[1, BLK]]))
    nc.vector.tensor_copy(scaleEr[:, n - 1:n], scaleA[:, 1:2])

    ph1.close()  # free ph1 tensors before the CKY working set

    es2 = contextlib.ExitStack()
    stage_pool = es2.enter_context(tc.tile_pool(name="stage", bufs=2))
    stageS_pool = es2.enter_context(tc.tile_pool(name="stageS", bufs=2))
    wash_pool = es2.enter_context(tc.tile_pool(name="wash", bufs=2))
    scr = es2.enter_context(tc.tile_pool(name="cky", bufs=2))
    scr1 = es2.enter_context(tc.tile_pool(name="cky1", bufs=1))

    # =======================================================================
    # Phase 4: CKY in scaled-exp space, fp16 data, integer pow2 scales.
    # chartA block L cols [L*BLK .. +72]: [0:36 chart | 40:56 FB | 56:72 FA]
    # chartE: same block layout at col (n-m)*BLK, rows indexed by span end.
    # scaleA col L / scaleEr col (n-m): integer scales as fp32.
    # true = stored * 2^scale;  G tables carry 2^GBOOST.
    # =======================================================================
    for L in range(2, n + 1):
        S = n - L + 1
        PS = 4 * S
        PSf = 4 * (S + 1)   # rows of the fresh block L-1
        NI = L - 2

        stageH = stage_pool.tile([128, (n + 1) * BLK], F16, tag="st")
        stageS = stageS_pool.tile([128, n + 2], F32, tag="sts")
        wash = wash_pool.tile([128, 1296], F16, tag="wa")

        # -- critical: fresh block L-1 (rows i+1 -> i), + its scale col.
        # chart cols [0:36] land first (unblock prodB); FA cols follow.
        nc.sync.dma_start(
            mk(stageH, PS, (n - L + 1) * BLK, [[1, 36]]),
            mk(chartA, PS, (L - 1) * BLK, [[1, 36]], base_part=4))
        nc.sync.dma_start(
            mk(stageS, PS, n - L + 1, [[1, 1]]),
            mk(scaleA, PS, L - 1, [[1, 1]], base_part=4))
        if NI > 0:
            nc.sync.dma_start(
                mk(stageH, PS, (n - L + 1) * BLK + 56, [[1, 16]]),
                mk(chartA, PS, (L - 1) * BLK + 56, [[1, 16]], base_part=4))
        # -- writeback fresh block L-1 to end-aligned history
        if L + 1 <= n:
            nc.sync.dma_start(
                mk(chartE, PSf, (n - L + 1) * BLK, [[1, BLK]],
                   base_part=4 * (L - 2)),
                mk(chartA, PSf, (L - 1) * BLK, [[1, BLK]]))
            nc.sync.dma_start(
                mk(scaleEr, PSf, n - L + 1, [[1, 1]], base_part=4 * (L - 2)),
                mk(scaleA, PSf, L - 1, [[1, 1]]))
        # -- prefetchable history: blocks 1..L-2 (+ scales)
        if L >= 3:
            nc.sync.dma_start(
                mk(stageH, PS, (n - L + 2) * BLK, [[1, (L - 2) * BLK]]),
                mk(chartE, PS, (n - L + 2) * BLK, [[1, (L - 2) * BLK]],
                   base_part=4 * (L - 1)))
            nc.sync.dma_start(
                mk(stageS, PS, n - L + 2, [[1, L - 2]]),
                mk(scaleEr, PS, n - L + 2, [[1, L - 2]],
                   base_part=4 * (L - 1)))
        # -- wash: WA rows at the right-end pair
        nc.sync.dma_start(
            mk(wash, PS, 0, [[1, 1296]]),
            mk(WA, PS, 0, [[1, 1296]], base_part=4 * (L - 1)))

        # ---- edge-A products (free of the critical DMAs)
        prodAB = scr1.tile([128, 2592], F16, tag="prod")
        nc.vector.tensor_tensor(
            prodAB[0:PS, 0:1296],
            mk(wash, PS, 0, [[1, 1296]]),
            mk(chartA, PS, (L - 1) * BLK, [[0, 36], [1, 36]]),
            op=ALU.mult)

        # ---- edge-B products (right after the fresh chart DMA lands)
        nc.vector.tensor_tensor(
            prodAB[0:PS, 1296:2592],
            mk(WB, PS, 0, [[1, 1296]]),
            mk(stageH, PS, (n - L + 1) * BLK, [[0, 36], [1, 36]]),
            op=ALU.mult)
        red72 = scr.tile([128, 160], F16, tag="red")
        nc.vector.tensor_reduce(red72[0:PS, 0:36],
                                mk(prodAB, PS, 0, [[36, 36], [1, 36]]),
                                axis=AXIS.X, op=ALU.add)
        nc.vector.tensor_reduce(red72[0:PS, 36:72],
                                mk(prodAB, PS, 1296, [[36, 36], [1, 36]]),
                                axis=AXIS.X, op=ALU.add)

        # ---- scales: sAsm[k] = sI_k, k=1..L-1; note the edge cases reuse
        # interior scales: sA = sI_{L-1} (k=L-1 split), sB = sI_1.
        sAsm = scr.tile([128, n + 8], F32, tag="sasm")
        eBits = scr.tile([128, n + 8], I32, tag="ebits")
        mstarN = scr.tile([128, 2], F32, tag="mstar")
        nc.vector.tensor_tensor(
            sAsm[0:PS, 1:L],
            scaleA[0:PS, 1:L],
            stageS[0:PS, n - L + 1:n], op=ALU.add)
        nc.vector.tensor_reduce(mstarN[0:PS, 0:1], sAsm[0:PS, 1:L],
                                axis=AXIS.X, op=ALU.max, negate=True)
        nc.vector.tensor_scalar(sAsm[0:PS, 1:L], sAsm[0:PS, 1:L],
                                mstarN[0:PS, 0:1], -126.0,
                                op0=ALU.add, op1=ALU.max)
        nc.vector.tensor_scalar(eBits[0:PS, 1:L], sAsm[0:PS, 1:L],
                                127.0, 8388608.0, op0=ALU.add, op1=ALU.mult)
        eAll = lambda off, dims: mk(eBits, PS, off, dims).bitcast(F32)

        # ---- interior terms (res<4), batched over k, on GpSimd
        if NI > 0:
            tI = scr1.tile([128, 2 * max(n - 2, 1) * 16], F16, tag="ti")
            nc.gpsimd.tensor_tensor(   # IA: chart[k][i] args x stage FA(L-k)
                mk(tI, PS, 0, [[2 * NI * 4, 4], [4, NI], [1, 4]]),
                mk(chartA, PS, BLK, [[0, 4], [BLK, NI], [1, 4]]),
                mk(stageH, PS, (n - L + 1) * BLK + 56,
                   [[4, 4], [BLK, NI], [1, 4]]),
                op=ALU.mult)
            nc.gpsimd.tensor_tensor(   # IB: stage args(L-k) x chart[k] FB
                mk(tI, PS, NI * 4, [[2 * NI * 4, 4], [4, NI], [1, 4]]),
                mk(stageH, PS, (n - L + 2) * BLK, [[0, 4], [BLK, NI], [1, 4]]),
                mk(chartA, PS, 2 * BLK + 40, [[4, 4], [BLK, NI], [1, 4]]),
                op=ALU.mult)
            for half in range(2):      # x eI (k scales), in place
                nc.gpsimd.tensor_tensor(
                    mk(tI, PS, half * NI * 4,
                       [[2 * NI * 4, 4], [4, NI], [1, 4]]),
                    mk(tI, PS, half * NI * 4,
                       [[2 * NI * 4, 4], [4, NI], [1, 4]]),
                    eAll(1 + half, [[0, 4], [1, NI], [0, 4]]),
                    op=ALU.mult)
            nc.gpsimd.tensor_tensor(   # fold IB half into IA half
                mk(tI, PS, 0, [[2 * NI * 4, 4], [4, NI], [1, 4]]),
                mk(tI, PS, 0, [[2 * NI * 4, 4], [4, NI], [1, 4]]),
                mk(tI, PS, NI * 4, [[2 * NI * 4, 4], [4, NI], [1, 4]]),
                op=ALU.add)

        # ---- combine cases with their pow2 scale factors (eA at col L-1,
        # eB at col 1: negative-stride pair view)
        nc.vector.tensor_tensor(red72[0:PS, 72:144], red72[0:PS, 0:72],
                                eAll(L - 1, [[-(L - 2), 2], [0, 36]]),
                                op=ALU.mult)
        tot36 = scr.tile([128, 40], F16, tag="tot")
        nc.vector.tensor_tensor(tot36[0:PS, 0:36], red72[0:PS, 72:108],
                                red72[0:PS, 108:144], op=ALU.add)
        if NI > 0:
            nc.vector.tensor_reduce(   # sum over (k, arg) -> [PS, 4]
                tot36[0:PS, 36:40],
                mk(tI, PS, 0,
                   [[2 * NI * 4, 4], [4, NI], [1, 4]]),
                axis=AXIS.XY, op=ALU.add)
            nc.vector.tensor_tensor(tot36[0:PS, 0:4], tot36[0:PS, 0:4],
                                    tot36[0:PS, 36:40], op=ALU.add)

        # ---- pow2 renorm and write chart block L
        # scale carries a +(127+GBOOST) per-level offset, removed at the end
        mvE = scr.tile([128, 4], F32, tag="mval")
        mvI = lambda c: mvE[0:PS, c:c + 1].bitcast(I32)
        nc.vector.tensor_reduce(mvE[0:PS, 0:1], tot36[0:PS, 0:36],
                                axis=AXIS.X, op=ALU.max)
        nc.vector.tensor_scalar(mvI(2), mvI(0), expMask[0:PS, 0:1], None,
                                op0=ALU.bitwise_and)
        nc.vector.reciprocal(mvE[0:PS, 1:2], mvE[0:PS, 2:3])
        nc.vector.tensor_scalar(
            chartA[0:PS, L * BLK:L * BLK + 36],
            tot36[0:PS, 0:36], mvE[0:PS, 1:2], None, op0=ALU.mult)
        nc.vector.tensor_scalar(scaleA[0:PS, L:L + 1], mvI(2),
                                1.0 / 8388608.0, mstarN[0:PS, 0:1],
                                op0=ALU.mult, op1=ALU.subtract)
        nc.gpsimd.tensor_tensor(
            mk(chartA, PS, L * BLK + 40, [[16, 2], [4, 4], [1, 4]]),
            mk(chartA, PS, L * BLK + 4, [[16, 2], [4, 4], [1, 4]]),
            mk(gfb, PS, 0, [[16, 2], [4, 4], [1, 4]]),
            op=ALU.mult)

    # =======================================================================
    # Phase 5: root -> nll per sentence
    # =======================================================================
    nc.vector.tensor_tensor(finh[:, 0:4],
                            mk(chartA, 4, n * BLK, [[1, 4]]),
                            rsRep[:], op=ALU.mult)
    nc.vector.tensor_reduce(fin[:, 4:5], finh[:, 0:4], axis=AXIS.X,
                            op=ALU.add)
    nc.scalar.activation(fin[:, 5:6], fin[:, 4:5], ACTF.Ln)
    # nll = -(ln(fin) + scale * ln2); scale carries +(127+GBOOST)(n-1)
    nc.vector.scalar_tensor_tensor(
        fin[:, 6:7], scaleA[0:4, n:n + 1], -LN2,
        fin[:, 5:6], op0=ALU.mult, op1=ALU.subtract)
    nc.vector.tensor_scalar_add(fin[:, 7:8], fin[:, 6:7],
                                float((127 + GBOOST) * (n - 1)) * LN2)
    nc.sync.dma_start(d["out"][:], fin[:, 7:8])
    es2.close()
    es.close()


# ============================================================== host wrapper
_PROG_CACHE = {}


def _get_program(cfg: Cfg):
    key = (cfg.n, cfg.v_loc, cfg.n_cores)
    if key not in _PROG_CACHE:
        _PROG_CACHE[key] = build_program(cfg)
    return _PROG_CACHE[key]


def make_inmaps(cfg: Cfg, inputs):
    """Host-side shard/pack of FULL inputs -> per-core DRAM input dicts."""
    import ml_dtypes
    bf16 = ml_dtypes.bfloat16
    x = np.asarray(inputs["x"])
    check_functor_tables(np.asarray(inputs["l_functors"]),
                         np.asarray(inputs["r_functors"]))
    nt_emb = np.asarray(inputs["nt_emb"], np.float32)          # [C, D]
    vocab_W = np.asarray(inputs["vocab_W"], np.float32)        # [D, V]
    vocab_b = np.asarray(inputs["vocab_b"], np.float32)        # [V]

    ntembT = np.zeros((65, CP), bf16)
    ntembT[0:64, 0:C] = nt_emb.T.astype(bf16)
    ntembT[64, :] = 1.0

    mlpW = np.zeros((64, 322), bf16)
    for j, k in enumerate(("sW1", "r1W1", "r1W2", "r2W1", "r2W2")):
        mlpW[:, j * 64:(j + 1) * 64] = np.asarray(inputs[k], np.float32)
    mlpW[:, 320:322] = np.asarray(inputs["sW2"], np.float32)

    mlpB = np.zeros((64, 8), np.float32)
    for j, k in enumerate(("sb1", "r1b1", "r1b2", "r2b1", "r2b2")):
        mlpB[:, j] = np.asarray(inputs[k], np.float32)

    ruleWb = np.zeros((36, 144), np.float32)
    ruleWb[:, 0:72] = np.asarray(inputs["rule_W"], np.float32)
    ruleWb[:, 72:144] = np.tile(
        np.asarray(inputs["rule_b"], np.float32)[None, :], (36, 1))

    smallv = np.zeros((1, 16), np.float32)
    smallv[0, 0:2] = np.asarray(inputs["sb2"], np.float32)
    smallv[0, 2:6] = np.asarray(inputs["root_W"], np.float32)[0, 0:4]
    smallv[0, 6:10] = np.asarray(inputs["root_b"], np.float32)[0:4]

    vs = cfg.v_loc
    in_maps = []
    for core in range(cfg.n_cores):
        vocabW = np.zeros((65, cfg.v_pad), bf16)
        vocabW[64, :] = NEGB
        vocabW[0:64, 0:vs] = vocab_W[:, core * vs:(core + 1) * vs]
        vocabW[64, 0:vs] = vocab_b[core * vs:(core + 1) * vs]

        words = x[core * BLOC:(core + 1) * BLOC, 0:cfg.n]   # [BLOC, n]
        wid = words.T.reshape(-1)                           # pair = i*4 + b
        wordW = np.zeros((66, cfg.pairs), bf16)
        wordW[0:64, :] = vocab_W[:, wid].astype(bf16)
        wordW[64, :] = vocab_b[wid]
        wordW[65, :] = 1.0

        in_maps.append({
            "ntembT": ntembT, "vocabW": vocabW, "wordW": wordW,
            "mlpW": mlpW, "mlpB": mlpB, "ruleWb": ruleWb, "smallv": smallv,
        })
    return in_maps


def kernel(**inputs) -> np.ndarray:
    cfg = Cfg(n=32, v_loc=V // NCORES, n_cores=NCORES)
    nc = _get_program(cfg)
    in_maps = make_inmaps(cfg, inputs)
    res = bass_utils.run_bass_kernel_spmd(
        nc, in_maps, core_ids=list(range(cfg.n_cores)))
    out = np.concatenate([r["out_nll"].reshape(-1) for r in res.results])
    return out.astype(np.float32)


if __name__ == "__main__":
    from reference import setup_inputs, reference
    inputs = {k: np.asarray(v) for k, v in setup_inputs().items()}
    got = kernel(**inputs)
    exp = np.asarray(reference(**inputs))
    rel = np.max(np.abs(got - exp) / np.maximum(np.abs(exp), 1e-6))
    print("expected:", exp[:8])
    print("got     :", got[:8])
    print("Relative error:", rel)
